# revision 1
# baseline (speedup 1.0000x reference)
"""Distributed Bass kernel for a 4-layer GAT autoencoder on 8 TRN2 NeuronCores.

Strategy (per sharding hint): nodes sharded across 8 cores (2500/core);
edges co-located with their destination node's core, sorted by destination;
params replicated.

v3 layout: the node phase is SHARDED — each core transforms only its own
2500 rows (inputs pre-sliced per core host-side, so all node-phase DMA uses
static local offsets), then the compact bf16 per-node tables [h | s_src]
are AllGathered so the edge phase can gather any source row. s_dst stays
local (a [2500,8] buffer). Pad edges carry an out-of-range dstloc (999), so
their one-hot column is all zeros and they contribute nothing to the
scatter or the softmax denominator — no NEGBIG dummy-row machinery.

Edge phase per dst tile: one-hot O for all chunks in one DVE op; a big bf16
dma_gather of source rows; per-edge s_dst via PE transpose(O_j) + an
8-column matmul; batched LeakyReLU/exp; per-chunk h*ex one-hot scatter
matmuls accumulating numerator and denominator in PSUM. BN statistics are
per-core partial sums (closed-group matmuls + SBUF accumulate) + a [128,2]
AllReduce; there is no separate stats pass.
"""

import sys

sys.path.insert(0, "/opt/trn_rl_repo")

import numpy as np

P = 128
M = 8
N = 20000
NPC = N // M  # 2500 nodes per core
NT = (NPC + P - 1) // P  # 20 dst tiles per core
HEADS = 8
NEG = 0.2
BN_EPS = 1e-5
PADDST = 999.0  # out-of-range dst slot for pad edges -> zero one-hot column

# layer configs: Fin, C (per-head out), concat?, bn on input?, bf16 row width R2
LAYERS = [
    dict(Fin=64, C=16, concat=True, bn=False, R2=256),
    dict(Fin=128, C=32, concat=False, bn=True, R2=384),
    dict(Fin=32, C=16, concat=True, bn=False, R2=256),
    dict(Fin=128, C=64, concat=False, bn=True, R2=640),
]
OWNW = [128, 32, 128, 64]  # own[l] row widths


def _wrap16(idx):
    """Host int array -> dma_gather index layout [16, n/16] (idx[s*16+p] at [p,s])."""
    n = idx.shape[0]
    assert n % 16 == 0
    w = np.ascontiguousarray(idx.reshape(n // 16, 16).T).astype(np.int16)
    return np.ascontiguousarray(np.tile(w, (8, 1)))  # replicated for the 8 Q7 cores


def _preprocess(edge_index):
    """Partition + sort edges; per-tile chunk counts; per-core gather indices."""
    src = np.concatenate([np.asarray(edge_index[0]), np.arange(N)]).astype(np.int64)
    dst = np.concatenate([np.asarray(edge_index[1]), np.arange(N)]).astype(np.int64)

    per_core = []
    cnts = np.zeros((M, NT), dtype=np.int64)
    for m in range(M):
        sel = (dst // NPC) == m
        s, d = src[sel], dst[sel]
        dloc = d - NPC * m
        order = np.argsort(dloc, kind="stable")
        s, dloc = s[order], dloc[order]
        tiles = []
        for t in range(NT):
            tsel = (dloc // P) == t
            st, dt_ = s[tsel], dloc[tsel] - t * P
            tiles.append((st, dt_))
            cnts[m, t] = st.shape[0]
        per_core.append(tiles)

    # per-tile chunk count: max over cores, rounded up to a multiple of 4
    # (keeps every tile's idx segment 64B-aligned for the dma_gather ucode)
    nch = np.maximum(((cnts.max(axis=0) + P - 1) // P + 3) // 4 * 4, 4)
    NCHT = [int(v) for v in nch]
    EPTT = [v * P for v in NCHT]
    TOT = int(sum(EPTT))

    data = []
    for m in range(M):
        isrc = np.zeros((TOT,), dtype=np.int64)  # pad edges gather row 0
        dloc_cols = np.full((P, sum(NCHT)), PADDST, dtype=np.float32)
        eoff = 0
        coff = 0
        for t in range(NT):
            st, dt_ = per_core[m][t]
            c = st.shape[0]
            isrc[eoff : eoff + c] = st
            dl = np.full((EPTT[t],), PADDST, dtype=np.float64)
            dl[:c] = dt_
            # column coff+j, row p  = edge (t, j*128+p)
            dloc_cols[:, coff : coff + NCHT[t]] = dl.reshape(NCHT[t], P).T
            eoff += EPTT[t]
            coff += NCHT[t]
        data.append(
            dict(idx_src=_wrap16(isrc), dstloc=np.ascontiguousarray(dloc_cols))
        )
    return NCHT, data


def _host_consts(inputs):
    """Fused weights + broadcast biases + misc consts."""
    f32 = np.float32
    c = {}
    c["iotab"] = np.tile(np.arange(P, dtype=f32)[None, :], (P, 1))
    c["ident"] = np.eye(P, dtype=f32)
    c["ones"] = np.ones((P, 1), dtype=f32)

    def fuse(W, a_s, a_d):
        # WW = [W | W@blockdiag(a_src) | W@blockdiag(a_dst)]  -> [Fin, HC+16]
        H, C_ = a_s.shape
        Ws = np.einsum("fhc,hc->fh", W.reshape(-1, H, C_), a_s)
        Wd = np.einsum("fhc,hc->fh", W.reshape(-1, H, C_), a_d)
        return np.concatenate([W, Ws, Wd], axis=1).astype(f32)

    c["ww1"] = fuse(inputs["We1"], inputs["as_e1"], inputs["ad_e1"])
    c["ww2"] = fuse(inputs["We2"], inputs["as_e2"], inputs["ad_e2"])
    c["ww3"] = fuse(inputs["Wd1"], inputs["as_d1"], inputs["ad_d1"])
    c["ww4"] = fuse(inputs["Wd2"], inputs["as_d2"], inputs["ad_d2"])
    c["bb1"] = np.tile(inputs["b_e1"][None, :], (P, 1)).astype(f32)  # [128,128]
    c["bb2"] = np.tile(inputs["b_e2"][None, :], (P, 1)).astype(f32)  # [128,32]
    c["bb3"] = np.tile(inputs["b_d1"][None, :], (P, 1)).astype(f32)  # [128,128]
    c["bb4"] = np.tile(inputs["b_d2"][None, :], (P, 1)).astype(f32)  # [128,64]
    c["bn1g"] = inputs["bn1_g"].astype(f32).reshape(-1, 1)  # [128,1]
    c["bn1b"] = inputs["bn1_b"].astype(f32).reshape(-1, 1)
    c["bn2g"] = inputs["bn2_g"].astype(f32).reshape(-1, 1)
    c["bn2b"] = inputs["bn2_b"].astype(f32).reshape(-1, 1)
    return c


def _build(NCHT, repeat_k=1):
    from concourse import bacc, bass, mybir, tile

    f32 = mybir.dt.float32
    bf16 = mybir.dt.bfloat16
    i16 = mybir.dt.int16
    nc = bacc.Bacc(
        "TRN2",
        target_bir_lowering=False,
        debug=False,
        enable_asserts=False,
        num_devices=M,
    )

    EPTT = [v * P for v in NCHT]
    TOT = sum(EPTT)
    TOTC = sum(NCHT)
    EOFF = np.concatenate([[0], np.cumsum(EPTT)]).astype(int)
    COFF = np.concatenate([[0], np.cumsum(NCHT)]).astype(int)

    def din(name, shape, dtype=f32):
        return nc.dram_tensor(name, list(shape), dtype, kind="ExternalInput")

    xin = din("xin", (NPC, 64))  # per-core slice of x
    idx_src = din("idx_src", (128, TOT // 16), i16)
    dstloc = din("dstloc", (P, TOTC))
    iotab = din("iotab", (P, P))
    ident = din("ident", (P, P))
    ones = din("ones", (P, 1))
    ww = [din(f"ww{l + 1}", (LAYERS[l]["Fin"], HEADS * LAYERS[l]["C"] + 16)) for l in range(4)]
    bb = [
        din("bb1", (P, 128)),
        din("bb2", (P, 32)),
        din("bb3", (P, 128)),
        din("bb4", (P, 64)),
    ]
    bng = [None, din("bn1g", (128, 1)), None, din("bn2g", (128, 1))]
    bnb = [None, din("bn1b", (128, 1)), None, din("bn2b", (128, 1))]
    out_ext = nc.dram_tensor("out", [NPC, 64], f32, kind="ExternalOutput")

    with tile.TileContext(nc) as tc:
        with (
            tc.tile_pool(name="dram", bufs=1, space="DRAM") as dram,
            tc.tile_pool(name="const", bufs=1) as cpool,
            tc.tile_pool(name="work", bufs=3) as wpool,
            tc.tile_pool(name="gath", bufs=2) as gpool,
            tc.tile_pool(name="psum", bufs=2, space="PSUM") as ppool,
        ):
            # ---- internal DRAM (local) ----
            tabL = [
                dram.tile([NPC, LAYERS[l]["R2"]], bf16, tag=f"tabL{l}", name=f"tabL{l}")
                for l in range(4)
            ]
            sdo = dram.tile([NPC, 8], f32, tag="sdo", name="sdo")
            own = [
                dram.tile([NPC, OWNW[l]], f32, tag=f"own{l}", name=f"own{l}")
                for l in range(3)
            ]
            ostat = [
                dram.tile([128, 2], f32, tag="ostat0", name="ostat0"),
                None,
                dram.tile([128, 2], f32, tag="ostat2", name="ostat2"),
            ]

            def fresh_shared(rep):
                sfx = "" if rep == 0 else f"r{rep}"
                tabA = [
                    dram.tile([N, LAYERS[l]["R2"]], bf16, tag=f"tabA{l}{sfx}",
                              name=f"tabA{l}{sfx}", addr_space="Shared")
                    for l in range(4)
                ]
                gstat = [
                    dram.tile([128, 2], f32, tag=f"gs0{sfx}", name=f"gs0{sfx}", addr_space="Shared"),
                    None,
                    dram.tile([128, 2], f32, tag=f"gs2{sfx}", name=f"gs2{sfx}", addr_space="Shared"),
                ]
                return tabA, gstat

            # ---- consts to SBUF ----
            def load_const(ap, shape, dtype=f32, tag=None):
                t = cpool.tile(list(shape), dtype, tag=tag, name=tag)
                nc.sync.dma_start(out=t[:], in_=ap[:])
                return t

            iotaf_sb = load_const(iotab, (P, P), tag="iotaf")
            ident_sb = load_const(ident, (P, P), tag="ident")
            ones_sb = load_const(ones, (P, 1), tag="ones")
            isrc_sb = load_const(idx_src, (128, TOT // 16), i16, tag="isrc")
            dstloc_sb = load_const(dstloc, (P, TOTC), tag="dstloc")
            ww_sb = [
                load_const(ww[l], (LAYERS[l]["Fin"], HEADS * LAYERS[l]["C"] + 16), tag=f"ww{l}")
                for l in range(4)
            ]
            bb_sb = [
                load_const(bb[0], (P, 128), tag="bb0"),
                load_const(bb[1], (P, 32), tag="bb1"),
                load_const(bb[2], (P, 128), tag="bb2"),
                load_const(bb[3], (P, 64), tag="bb3"),
            ]
            bng_sb = [None, load_const(bng[1], (128, 1), tag="bng1"), None, load_const(bng[3], (128, 1), tag="bng3")]
            bnb_sb = [None, load_const(bnb[1], (128, 1), tag="bnb1"), None, load_const(bnb[3], (128, 1), tag="bnb3")]

            AX = mybir.AxisListType.X
            OP = mybir.AluOpType
            AF = mybir.ActivationFunctionType

            iotab_sb = cpool.tile([P, P], bf16, tag="iotabf", name="iotabf")
            nc.vector.tensor_copy(iotab_sb[:], iotaf_sb[:])
            identb_sb = cpool.tile([P, P], bf16, tag="identb", name="identb")
            nc.vector.tensor_copy(identb_sb[:], ident_sb[:])

            # registers holding num_idxs values for dma_gather
            ept_regs = {}
            for v in sorted(set(EPTT)):
                r = nc.alloc_registers(name=f"ept{v}")
                nc.regs_mov(r, v)
                ept_regs[v] = nc.snap(r, donate=False)

            # ============ node phase (own 2500 rows only) ============
            def node_phase(l, src_dram, gstat):
                cfg = LAYERS[l]
                Fin, C, R2 = cfg["Fin"], cfg["C"], cfg["R2"]
                HC = HEADS * C
                scale_off = None
                if cfg["bn"]:
                    sg = wpool.tile([128, 2], f32, tag="sg", name="sg")
                    nc.sync.dma_start(out=sg[:], in_=gstat[l - 1][:])
                    mu = wpool.tile([Fin, 1], f32, tag="mu", name="mu")
                    nc.vector.tensor_scalar(mu[:], sg[:, 0:1], 1.0 / N, None, OP.mult)
                    msq = wpool.tile([Fin, 1], f32, tag="msq", name="msq")
                    nc.vector.tensor_scalar(msq[:], sg[:, 1:2], 1.0 / N, None, OP.mult)
                    var = wpool.tile([Fin, 1], f32, tag="var", name="var")
                    nc.vector.tensor_tensor(var[:], mu[:], mu[:], OP.mult)
                    nc.vector.tensor_tensor(var[:], msq[:], var[:], OP.subtract)
                    nc.vector.tensor_scalar(var[:], var[:], BN_EPS, None, OP.add)
                    sdv = wpool.tile([Fin, 1], f32, tag="sdv", name="sdv")
                    nc.scalar.activation(sdv[:], var[:], AF.Sqrt)
                    rs = wpool.tile([Fin, 1], f32, tag="rs", name="rs")
                    nc.vector.reciprocal(rs[:], sdv[:])
                    bscale = wpool.tile([Fin, 1], f32, tag="bscale", name="bscale")
                    nc.vector.tensor_tensor(bscale[:], rs[:], bng_sb[l][:], OP.mult)
                    boff = wpool.tile([Fin, 1], f32, tag="boff", name="boff")
                    nc.vector.tensor_tensor(boff[:], mu[:], bscale[:], OP.mult)
                    nc.vector.tensor_tensor(boff[:], bnb_sb[l][:], boff[:], OP.subtract)
                    scale_off = (bscale, boff)

                for t in range(NT):
                    cnt = min(P, NPC - t * P)
                    xt = wpool.tile([P, Fin], f32, tag="xt", name="xt")
                    if cnt < P:
                        nc.vector.memset(xt[:], 0.0)
                    nc.sync.dma_start(out=xt[:cnt, :], in_=src_dram[t * P : t * P + cnt, :Fin])
                    xtp = ppool.tile([Fin, P], f32, tag="xtp", name="xtp", bufs=2)
                    nc.tensor.transpose(out=xtp[:], in_=xt[:], identity=ident_sb[:])
                    xts = wpool.tile([Fin, P], f32, tag="xts", name="xts")
                    if scale_off is not None:
                        nc.vector.tensor_scalar(
                            xts[:], xtp[:], scale_off[0][:], scale_off[1][:], OP.mult, OP.add
                        )
                        nc.scalar.activation(xts[:], xts[:], AF.Relu)
                    else:
                        nc.vector.tensor_copy(xts[:], xtp[:])
                    tt = wpool.tile([P, HC + 8], bf16, tag="tt", name="tt")
                    sd = wpool.tile([P, 8], f32, tag="sd", name="sd")
                    if HC + 16 <= 512:
                        hp = ppool.tile([P, HC + 16], f32, tag="pmm", name="hp", bufs=2)
                        nc.tensor.matmul(out=hp[:], lhsT=xts[:], rhs=ww_sb[l][:], start=True, stop=True)
                        nc.vector.tensor_copy(tt[:], hp[:, : HC + 8])
                        nc.vector.tensor_copy(sd[:], hp[:, HC + 8 : HC + 16])
                    else:  # L4: 528 cols -> split 512 + 16
                        hp = ppool.tile([P, 512], f32, tag="pmm", name="hp", bufs=2)
                        hp2 = ppool.tile([P, 16], f32, tag="pmm2", name="hp2", bufs=1)
                        nc.tensor.matmul(out=hp[:], lhsT=xts[:], rhs=ww_sb[l][:, :512], start=True, stop=True)
                        nc.tensor.matmul(out=hp2[:], lhsT=xts[:], rhs=ww_sb[l][:, 512:], start=True, stop=True)
                        nc.vector.tensor_copy(tt[:, :512], hp[:])
                        nc.vector.tensor_copy(tt[:, 512:520], hp2[:, 0:8])
                        nc.vector.tensor_copy(sd[:], hp2[:, 8:16])
                    nc.sync.dma_start(
                        out=tabL[l][t * P : t * P + cnt, : HC + 8], in_=tt[:cnt, :]
                    )
                    nc.sync.dma_start(out=sdo[t * P : t * P + cnt, :], in_=sd[:cnt, :])

            # ============ edge phase (own dst tiles) ============
            def edge_phase(l, tabA, out_dram):
                cfg = LAYERS[l]
                C, R2 = cfg["C"], cfg["R2"]
                HC = HEADS * C
                do_stat = l in (0, 2)
                if do_stat:
                    accS = wpool.tile([HC, 2], f32, tag="accS", name="accS")
                    nc.vector.memset(accS[:], 0.0)
                for t in range(NT):
                    NCH = NCHT[t]
                    EPT = EPTT[t]
                    cnt = min(P, NPC - t * P)
                    # one-hot O for all chunks of this tile (single DVE op);
                    # pad edges have dstloc=999 -> all-zero column
                    Oall = gpool.tile([P, NCH * P], bf16, tag="Oall", name="Oall")
                    nc.vector.tensor_tensor(
                        Oall[:].rearrange("p (j f) -> p j f", f=P),
                        iotab_sb[:].unsqueeze(1).to_broadcast((P, NCH, P)),
                        dstloc_sb[:, COFF[t] : COFF[t] + NCH].unsqueeze(2).to_broadcast((P, NCH, P)),
                        OP.is_equal,
                    )
                    # own s_dst rows for this tile (local, static offset)
                    sdtf = wpool.tile([P, 8], f32, tag="sdtf", name="sdtf")
                    if cnt < P:
                        nc.vector.memset(sdtf[:], 0.0)
                    nc.sync.dma_start(out=sdtf[:cnt, :], in_=sdo[t * P : t * P + cnt, :])
                    sdtb = wpool.tile([P, 8], bf16, tag="sdtb", name="sdtb")
                    nc.vector.tensor_copy(sdtb[:], sdtf[:])
                    # per-chunk SD = transpose(O_j).T @ sdt  (PE; overlaps gather)
                    psSD = ppool.tile([P, NCH * 8], f32, tag="psSD", name="psSD", bufs=1)
                    for j in range(NCH):
                        psOT = ppool.tile([P, P], bf16, tag="xtp", name="psOT", bufs=2)
                        nc.tensor.transpose(
                            out=psOT[:], in_=Oall[:, j * P : (j + 1) * P], identity=identb_sb[:]
                        )
                        OTs = wpool.tile([P, P], bf16, tag="OTs", name="OTs")
                        nc.scalar.activation(OTs[:], psOT[:], AF.Identity)
                        nc.tensor.matmul(
                            out=psSD[:, j * 8 : (j + 1) * 8], lhsT=OTs[:], rhs=sdtb[:],
                            start=True, stop=True,
                        )
                    # gather source rows [h | s_src] (bf16) from the global table
                    G = gpool.tile([P, NCH * R2], bf16, tag="G", name="G")
                    nc.gpsimd.dma_gather(
                        out_ap=G[:].rearrange("p (j r) -> p j r", r=R2),
                        in_ap=tabA[l][:],
                        idxs_ap=isrc_sb[:, EOFF[t] // 16 : EOFF[t + 1] // 16],
                        num_idxs=EPT,
                        num_idxs_reg=ept_regs[EPT],
                        elem_size=R2,
                        single_packet=False,
                    )
                    G3 = G[:].rearrange("p (j r) -> p j r", r=R2)
                    # batched e = LeakyReLU(s_src + s_dst); EXS = exp(e) (bf16)
                    EB = wpool.tile([P, NCH * 8], f32, tag="EB", name="EB")
                    nc.vector.tensor_copy(
                        EB[:].rearrange("p (j r) -> p j r", r=8), G3[:, :, HC : HC + 8]
                    )
                    nc.vector.tensor_tensor(EB[:], EB[:], psSD[:], OP.add)
                    EB2 = wpool.tile([P, NCH * 8], f32, tag="EB2", name="EB2")
                    nc.vector.tensor_scalar(EB2[:], EB[:], NEG, None, OP.mult)
                    nc.vector.tensor_tensor(EB[:], EB[:], EB2[:], OP.max)
                    EXS = wpool.tile([P, NCH * 8], bf16, tag="EXS", name="EXS")
                    nc.scalar.activation(EXS[:], EB[:], AF.Exp)
                    # per-chunk weighted scatter
                    if HC + 8 <= 512:
                        psA = ppool.tile([P, HC + 8], f32, tag="pmm", name="psA", bufs=2)
                        psB = None
                    else:
                        psA = ppool.tile([P, 512], f32, tag="pmm", name="psA", bufs=2)
                        psB = ppool.tile([P, 8], f32, tag="pmm2", name="psB", bufs=1)
                    for j in range(NCH):
                        GEX = wpool.tile([P, HC + 8], bf16, tag="GEX", name="GEX")
                        nc.vector.tensor_tensor(
                            GEX[:, :HC].rearrange("p (h c) -> p h c", h=HEADS),
                            G3[:, j, :HC].rearrange("p (h c) -> p h c", h=HEADS),
                            EXS[:, j * 8 : (j + 1) * 8].unsqueeze(2).to_broadcast((P, HEADS, C)),
                            OP.mult,
                        )
                        nc.gpsimd.tensor_copy(GEX[:, HC : HC + 8], EXS[:, j * 8 : (j + 1) * 8])
                        if psB is None:
                            nc.tensor.matmul(
                                out=psA[:], lhsT=Oall[:, j * P : (j + 1) * P], rhs=GEX[:],
                                start=(j == 0), stop=(j == NCH - 1),
                            )
                        else:
                            nc.tensor.matmul(
                                out=psA[:], lhsT=Oall[:, j * P : (j + 1) * P], rhs=GEX[:, :512],
                                start=(j == 0), stop=(j == NCH - 1),
                            )
                            nc.tensor.matmul(
                                out=psB[:], lhsT=Oall[:, j * P : (j + 1) * P], rhs=GEX[:, 512:],
                                start=(j == 0), stop=(j == NCH - 1),
                            )
                    den = psA[:, HC : HC + 8] if psB is None else psB[:]
                    rec = wpool.tile([P, 8], f32, tag="rec", name="rec")
                    nc.vector.tensor_scalar(rec[:], den, 1e-16, None, OP.add)
                    nc.vector.reciprocal(rec[:], rec[:])
                    res = wpool.tile([P, HC], f32, tag="res", name="res")
                    nc.vector.tensor_tensor(
                        res[:].rearrange("p (h c) -> p h c", h=HEADS),
                        psA[:, :HC].rearrange("p (h c) -> p h c", h=HEADS),
                        rec[:].unsqueeze(2).to_broadcast((P, HEADS, C)),
                        OP.mult,
                    )
                    if cfg["concat"]:
                        nc.vector.tensor_tensor(res[:], res[:], bb_sb[l][:], OP.add)
                        nc.sync.dma_start(
                            out=out_dram[t * P : t * P + cnt, :], in_=res[:cnt, :]
                        )
                        if do_stat:
                            sq = wpool.tile([P, HC], f32, tag="sq", name="sq")
                            nc.scalar.square(sq[:], res[:])
                            psS1 = ppool.tile([HC, 1], f32, tag="psS", name="psS1", bufs=2)
                            nc.tensor.matmul(
                                out=psS1[:], lhsT=res[:cnt, :], rhs=ones_sb[:cnt, :],
                                start=True, stop=True,
                            )
                            nc.vector.tensor_tensor(accS[:, 0:1], accS[:, 0:1], psS1[:], OP.add)
                            psS2 = ppool.tile([HC, 1], f32, tag="psS", name="psS2", bufs=2)
                            nc.tensor.matmul(
                                out=psS2[:], lhsT=sq[:cnt, :], rhs=ones_sb[:cnt, :],
                                start=True, stop=True,
                            )
                            nc.vector.tensor_tensor(accS[:, 1:2], accS[:, 1:2], psS2[:], OP.add)
                    else:
                        red = wpool.tile([P, C], f32, tag="red", name="red")
                        nc.vector.tensor_reduce(
                            red[:],
                            res[:].rearrange("p (h c) -> p c h", h=HEADS),
                            AX,
                            OP.add,
                        )
                        nc.vector.tensor_scalar(red[:], red[:], 1.0 / HEADS, None, OP.mult)
                        nc.vector.tensor_tensor(red[:], red[:], bb_sb[l][:, :C], OP.add)
                        nc.sync.dma_start(
                            out=out_dram[t * P : t * P + cnt, :], in_=red[:cnt, :]
                        )
                if do_stat:
                    nc.sync.dma_start(out=ostat[l][:], in_=accS[:])

            # ================= full pipeline =================
            for _rep in range(repeat_k):
                tabA, gstat = fresh_shared(_rep)
                srcs = [xin, own[0], own[1], own[2]]
                outs = [own[0], own[1], own[2], out_ext]
                for l in range(4):
                    node_phase(l, srcs[l], gstat)
                    nc.gpsimd.collective_compute(
                        "AllGather",
                        mybir.AluOpType.bypass,
                        replica_groups=[list(range(M))],
                        ins=[tabL[l].opt()],
                        outs=[tabA[l].opt()],
                    )
                    edge_phase(l, tabA, outs[l])
                    if l in (0, 2):
                        nc.gpsimd.collective_compute(
                            "AllReduce",
                            mybir.AluOpType.add,
                            replica_groups=[list(range(M))],
                            ins=[ostat[l].opt()],
                            outs=[gstat[l].opt()],
                        )
    if not nc.is_finalized():
        nc.finalize()
    return nc


def _pjrt_exec(nc, in_maps, time_reps=0):
    """Mirror of bass2jax.run_bass_via_pjrt multi-core path, holding the jitted
    executable so repeated executions can be wall-timed."""
    import time as _t
    import jax
    from jax.experimental.shard_map import shard_map
    from jax.sharding import Mesh, PartitionSpec
    from concourse import bass2jax as B, mybir as mb

    B.install_neuronx_cc_hook()
    n_cores = len(in_maps)
    partition_name = nc.partition_id_tensor.name if nc.partition_id_tensor else None
    in_names, out_names, out_avals, zero_outs = [], [], [], []
    for alloc in nc.m.functions[0].allocations:
        if not isinstance(alloc, mb.MemoryLocationSet):
            continue
        name = alloc.memorylocations[0].name
        if alloc.kind == "ExternalInput":
            if name != partition_name:
                in_names.append(name)
        elif alloc.kind == "ExternalOutput":
            out_names.append(name)
            shape = tuple(alloc.tensor_shape)
            dtype = mb.dt.np(alloc.dtype)
            out_avals.append(jax.core.ShapedArray(shape, dtype))
            zero_outs.append(np.zeros(shape, dtype))
    n_params = len(in_names)
    n_outs = len(out_avals)
    in_names.extend(out_names)
    if partition_name is not None:
        in_names.append(partition_name)
    donate = tuple(range(n_params, n_params + n_outs))

    def _body(*args):
        operands = list(args)
        if partition_name is not None:
            operands.append(B.partition_id_tensor())
        outs = B._bass_exec_p.bind(
            *operands,
            out_avals=tuple(out_avals),
            in_names=tuple(in_names),
            out_names=tuple(out_names),
            lowering_input_output_aliases=(),
            sim_require_finite=True,
            sim_require_nnan=True,
            nc=nc,
        )
        return tuple(outs)

    devices = jax.devices()[:n_cores]
    mesh = Mesh(np.asarray(devices), ("core",))
    in_specs = (PartitionSpec("core"),) * (n_params + n_outs)
    out_specs = (PartitionSpec("core"),) * len(out_names)
    sharded = jax.jit(
        shard_map(_body, mesh=mesh, in_specs=in_specs, out_specs=out_specs,
                  check_rep=False),
        donate_argnums=donate, keep_unused=True,
    )
    per_core = [[np.asarray(m_[nm]) for nm in in_names[:n_params]] for m_ in in_maps]
    concat_in = [
        np.concatenate([per_core[c][i] for c in range(n_cores)], axis=0)
        for i in range(n_params)
    ]
    from jax.sharding import NamedSharding
    shard = NamedSharding(mesh, PartitionSpec("core"))
    concat_in = [jax.device_put(a, shard) for a in concat_in]
    jax.block_until_ready(concat_in)

    def once():
        cz = [jax.device_put(np.zeros((n_cores * z.shape[0], *z.shape[1:]), z.dtype), shard)
              for z in zero_outs]
        jax.block_until_ready(cz)
        t0 = _t.perf_counter()
        out_arrs = sharded(*concat_in, *cz)
        jax.block_until_ready(out_arrs)
        return _t.perf_counter() - t0, out_arrs

    _, out_arrs = once()  # compile + first run
    times = []
    for _ in range(time_reps):
        dt, out_arrs = once()
        times.append(dt)
    res = [
        {nm: np.asarray(out_arrs[i]).reshape(n_cores, *out_avals[i].shape)[c]
         for i, nm in enumerate(out_names)}
        for c in range(n_cores)
    ]
    return res, (min(times) if times else None)


def _run(inputs, trace=False, time_reps=0, repeat_k=1):
    NCHT, edata = _preprocess(np.asarray(inputs["edge_index"]))
    consts = _host_consts(inputs)
    nc = _build(NCHT, repeat_k=repeat_k)

    x = np.asarray(inputs["x"], dtype=np.float32)
    in_maps = []
    for m in range(M):
        d = dict(consts)
        d.update(edata[m])
        d["xin"] = np.ascontiguousarray(x[m * NPC : (m + 1) * NPC])
        in_maps.append(d)

    if time_reps > 0:
        results, best_s = _pjrt_exec(nc, in_maps, time_reps=time_reps)
    else:
        from concourse.bass_utils import run_bass_kernel_spmd

        res = run_bass_kernel_spmd(nc, in_maps, core_ids=list(range(M)))
        results, best_s = res.results, None
    outs = [np.asarray(results[m]["out"]) for m in range(M)]
    full = np.concatenate(outs, axis=0).astype(np.float32)
    return full, (None if best_s is None else int(best_s * 1e9))


def kernel(**inputs):
    out, _ = _run(inputs, trace=False)
    return out



# revision 5
# speedup vs baseline: 24.0641x; 24.0641x over previous
"""Distributed Bass kernel for a 4-layer GAT autoencoder on 8 TRN2 NeuronCores.

Strategy (per sharding hint): nodes sharded across 8 cores (2500/core);
edges co-located with their destination node's core, sorted by destination;
params replicated.

v3 layout: the node phase is SHARDED — each core transforms only its own
2500 rows (inputs pre-sliced per core host-side, so all node-phase DMA uses
static local offsets), then the compact bf16 per-node tables [h | s_src]
are AllGathered so the edge phase can gather any source row. s_dst stays
local (a [2500,8] buffer). Pad edges carry an out-of-range dstloc (999), so
their one-hot column is all zeros and they contribute nothing to the
scatter or the softmax denominator — no NEGBIG dummy-row machinery.

Edge phase per dst tile: one-hot O for all chunks in one DVE op; a big bf16
dma_gather of source rows; per-edge s_dst via PE transpose(O_j) + an
8-column matmul; batched LeakyReLU/exp; per-chunk h*ex one-hot scatter
matmuls accumulating numerator and denominator in PSUM. BN statistics are
per-core partial sums (closed-group matmuls + SBUF accumulate) + a [128,2]
AllReduce; there is no separate stats pass.
"""

import sys

sys.path.insert(0, "/opt/trn_rl_repo")

import numpy as np

P = 128
M = 8
N = 20000
NPC = N // M  # 2500 nodes per core
NT = (NPC + P - 1) // P  # 20 dst tiles per core
HEADS = 8
NEG = 0.2
BN_EPS = 1e-5
PADDST = 999.0  # out-of-range dst slot for pad edges -> zero one-hot column

# layer configs: Fin, C (per-head out), concat?, bn on input?, bf16 row width R2
LAYERS = [
    dict(Fin=64, C=16, concat=True, bn=False, R2=256),
    dict(Fin=128, C=32, concat=False, bn=True, R2=384),
    dict(Fin=32, C=16, concat=True, bn=False, R2=256),
    dict(Fin=128, C=64, concat=False, bn=True, R2=640),
]
OWNW = [128, 32, 128, 64]  # own[l] row widths


def _wrap16(idx):
    """Host int array -> dma_gather index layout [16, n/16] (idx[s*16+p] at [p,s])."""
    n = idx.shape[0]
    assert n % 16 == 0
    w = np.ascontiguousarray(idx.reshape(n // 16, 16).T).astype(np.int16)
    return np.ascontiguousarray(np.tile(w, (8, 1)))  # replicated for the 8 Q7 cores


def _preprocess(edge_index):
    """Partition + sort edges; per-tile chunk counts; per-core gather indices."""
    src = np.concatenate([np.asarray(edge_index[0]), np.arange(N)]).astype(np.int64)
    dst = np.concatenate([np.asarray(edge_index[1]), np.arange(N)]).astype(np.int64)

    per_core = []
    cnts = np.zeros((M, NT), dtype=np.int64)
    for m in range(M):
        sel = (dst // NPC) == m
        s, d = src[sel], dst[sel]
        dloc = d - NPC * m
        order = np.argsort(dloc, kind="stable")
        s, dloc = s[order], dloc[order]
        tiles = []
        for t in range(NT):
            tsel = (dloc // P) == t
            st, dt_ = s[tsel], dloc[tsel] - t * P
            tiles.append((st, dt_))
            cnts[m, t] = st.shape[0]
        per_core.append(tiles)

    # per-tile chunk count: max over cores, rounded up to a multiple of 4
    # (keeps every tile's idx segment 64B-aligned for the dma_gather ucode)
    nch = np.maximum(((cnts.max(axis=0) + P - 1) // P + 3) // 4 * 4, 4)
    NCHT = [int(v) for v in nch]
    EPTT = [v * P for v in NCHT]
    TOT = int(sum(EPTT))

    data = []
    for m in range(M):
        isrc = np.zeros((TOT,), dtype=np.int64)  # pad edges gather row 0
        dloc_cols = np.full((P, sum(NCHT)), PADDST, dtype=np.float32)
        eoff = 0
        coff = 0
        for t in range(NT):
            st, dt_ = per_core[m][t]
            c = st.shape[0]
            isrc[eoff : eoff + c] = st
            dl = np.full((EPTT[t],), PADDST, dtype=np.float64)
            dl[:c] = dt_
            # column coff+j, row p  = edge (t, j*128+p)
            dloc_cols[:, coff : coff + NCHT[t]] = dl.reshape(NCHT[t], P).T
            eoff += EPTT[t]
            coff += NCHT[t]
        data.append(
            dict(idx_src=_wrap16(isrc), dstloc=np.ascontiguousarray(dloc_cols))
        )
    return NCHT, data


def _host_consts(inputs):
    """Fused weights + broadcast biases + misc consts."""
    f32 = np.float32
    c = {}
    c["iotab"] = np.tile(np.arange(P, dtype=f32)[None, :], (P, 1))
    c["ident"] = np.eye(P, dtype=f32)
    c["ones"] = np.ones((P, 1), dtype=f32)

    def fuse(W, a_s, a_d):
        # WW = [W | W@blockdiag(a_src) | W@blockdiag(a_dst)]  -> [Fin, HC+16]
        H, C_ = a_s.shape
        Ws = np.einsum("fhc,hc->fh", W.reshape(-1, H, C_), a_s)
        Wd = np.einsum("fhc,hc->fh", W.reshape(-1, H, C_), a_d)
        return np.concatenate([W, Ws, Wd], axis=1).astype(f32)

    c["ww1"] = fuse(inputs["We1"], inputs["as_e1"], inputs["ad_e1"])
    c["ww2"] = fuse(inputs["We2"], inputs["as_e2"], inputs["ad_e2"])
    c["ww3"] = fuse(inputs["Wd1"], inputs["as_d1"], inputs["ad_d1"])
    c["ww4"] = fuse(inputs["Wd2"], inputs["as_d2"], inputs["ad_d2"])
    c["bb1"] = np.tile(inputs["b_e1"][None, :], (P, 1)).astype(f32)  # [128,128]
    c["bb2"] = np.tile(inputs["b_e2"][None, :], (P, 1)).astype(f32)  # [128,32]
    c["bb3"] = np.tile(inputs["b_d1"][None, :], (P, 1)).astype(f32)  # [128,128]
    c["bb4"] = np.tile(inputs["b_d2"][None, :], (P, 1)).astype(f32)  # [128,64]
    c["bn1g"] = inputs["bn1_g"].astype(f32).reshape(-1, 1)  # [128,1]
    c["bn1b"] = inputs["bn1_b"].astype(f32).reshape(-1, 1)
    c["bn2g"] = inputs["bn2_g"].astype(f32).reshape(-1, 1)
    c["bn2b"] = inputs["bn2_b"].astype(f32).reshape(-1, 1)
    return c


def _build(NCHT, repeat_k=1, single_core=False):
    from concourse import bacc, bass, mybir, tile

    f32 = mybir.dt.float32
    bf16 = mybir.dt.bfloat16
    i16 = mybir.dt.int16
    nc = bacc.Bacc(
        "TRN2",
        target_bir_lowering=False,
        debug=False,
        enable_asserts=False,
        num_devices=1 if single_core else M,
    )

    EPTT = [v * P for v in NCHT]
    TOT = sum(EPTT)
    TOTC = sum(NCHT)
    EOFF = np.concatenate([[0], np.cumsum(EPTT)]).astype(int)
    COFF = np.concatenate([[0], np.cumsum(NCHT)]).astype(int)

    def din(name, shape, dtype=f32):
        return nc.dram_tensor(name, list(shape), dtype, kind="ExternalInput")

    xin = din("xin", (NPC, 64))  # per-core slice of x
    idx_src = din("idx_src", (128, TOT // 16), i16)
    dstloc = din("dstloc", (P, TOTC))
    iotab = din("iotab", (P, P))
    ident = din("ident", (P, P))
    ones = din("ones", (P, 1))
    ww = [din(f"ww{l + 1}", (LAYERS[l]["Fin"], HEADS * LAYERS[l]["C"] + 16)) for l in range(4)]
    bb = [
        din("bb1", (P, 128)),
        din("bb2", (P, 32)),
        din("bb3", (P, 128)),
        din("bb4", (P, 64)),
    ]
    bng = [None, din("bn1g", (128, 1)), None, din("bn2g", (128, 1))]
    bnb = [None, din("bn1b", (128, 1)), None, din("bn2b", (128, 1))]
    out_ext = nc.dram_tensor("out", [NPC, 64], f32, kind="ExternalOutput")

    with tile.TileContext(nc) as tc:
        with (
            tc.tile_pool(name="dram", bufs=1, space="DRAM") as dram,
            tc.tile_pool(name="const", bufs=1) as cpool,
            tc.tile_pool(name="work", bufs=3) as wpool,
            tc.tile_pool(name="gath", bufs=2) as gpool,
            tc.tile_pool(name="psum", bufs=2, space="PSUM") as ppool,
        ):
            # ---- internal DRAM (local) ----
            tabL = [
                dram.tile([NPC, LAYERS[l]["R2"]], bf16, tag=f"tabL{l}", name=f"tabL{l}")
                for l in range(4)
            ]
            sdo = dram.tile([NPC, 8], f32, tag="sdo", name="sdo")
            own = [
                dram.tile([NPC, OWNW[l]], f32, tag=f"own{l}", name=f"own{l}")
                for l in range(3)
            ]
            ostat = [
                dram.tile([128, 2], f32, tag="ostat0", name="ostat0"),
                None,
                dram.tile([128, 2], f32, tag="ostat2", name="ostat2"),
            ]

            def fresh_shared(rep):
                sfx = "" if rep == 0 else f"r{rep}"
                aspace = {} if single_core else dict(addr_space="Shared")
                tabA = [
                    dram.tile([N, LAYERS[l]["R2"]], bf16, tag=f"tabA{l}{sfx}",
                              name=f"tabA{l}{sfx}", **aspace)
                    for l in range(4)
                ]
                gstat = [
                    dram.tile([128, 2], f32, tag=f"gs0{sfx}", name=f"gs0{sfx}", **aspace),
                    None,
                    dram.tile([128, 2], f32, tag=f"gs2{sfx}", name=f"gs2{sfx}", **aspace),
                ]
                return tabA, gstat

            # ---- consts to SBUF ----
            def load_const(ap, shape, dtype=f32, tag=None):
                t = cpool.tile(list(shape), dtype, tag=tag, name=tag)
                nc.sync.dma_start(out=t[:], in_=ap[:])
                return t

            iotaf_sb = load_const(iotab, (P, P), tag="iotaf")
            ident_sb = load_const(ident, (P, P), tag="ident")
            ones_sb = load_const(ones, (P, 1), tag="ones")
            isrc_sb = load_const(idx_src, (128, TOT // 16), i16, tag="isrc")
            dstloc_sb = load_const(dstloc, (P, TOTC), tag="dstloc")
            ww_sb = [
                load_const(ww[l], (LAYERS[l]["Fin"], HEADS * LAYERS[l]["C"] + 16), tag=f"ww{l}")
                for l in range(4)
            ]
            bb_sb = [
                load_const(bb[0], (P, 128), tag="bb0"),
                load_const(bb[1], (P, 32), tag="bb1"),
                load_const(bb[2], (P, 128), tag="bb2"),
                load_const(bb[3], (P, 64), tag="bb3"),
            ]
            bng_sb = [None, load_const(bng[1], (128, 1), tag="bng1"), None, load_const(bng[3], (128, 1), tag="bng3")]
            bnb_sb = [None, load_const(bnb[1], (128, 1), tag="bnb1"), None, load_const(bnb[3], (128, 1), tag="bnb3")]

            AX = mybir.AxisListType.X
            OP = mybir.AluOpType
            AF = mybir.ActivationFunctionType

            iotab_sb = cpool.tile([P, P], bf16, tag="iotabf", name="iotabf")
            nc.vector.tensor_copy(iotab_sb[:], iotaf_sb[:])
            identb_sb = cpool.tile([P, P], bf16, tag="identb", name="identb")
            nc.vector.tensor_copy(identb_sb[:], ident_sb[:])

            # registers holding num_idxs values for dma_gather
            ept_regs = {}
            for v in sorted(set(EPTT)):
                r = nc.alloc_registers(name=f"ept{v}")
                nc.regs_mov(r, v)
                ept_regs[v] = nc.snap(r, donate=False)

            # ============ node phase (own 2500 rows only) ============
            def node_phase(l, src_dram, gstat):
                cfg = LAYERS[l]
                Fin, C, R2 = cfg["Fin"], cfg["C"], cfg["R2"]
                HC = HEADS * C
                scale_off = None
                if cfg["bn"]:
                    sg = wpool.tile([128, 2], f32, tag="sg", name="sg")
                    nc.sync.dma_start(out=sg[:], in_=gstat[l - 1][:])
                    mu = wpool.tile([Fin, 1], f32, tag="mu", name="mu")
                    nc.vector.tensor_scalar(mu[:], sg[:, 0:1], 1.0 / N, None, OP.mult)
                    msq = wpool.tile([Fin, 1], f32, tag="msq", name="msq")
                    nc.vector.tensor_scalar(msq[:], sg[:, 1:2], 1.0 / N, None, OP.mult)
                    var = wpool.tile([Fin, 1], f32, tag="var", name="var")
                    nc.vector.tensor_tensor(var[:], mu[:], mu[:], OP.mult)
                    nc.vector.tensor_tensor(var[:], msq[:], var[:], OP.subtract)
                    nc.vector.tensor_scalar(var[:], var[:], BN_EPS, None, OP.add)
                    sdv = wpool.tile([Fin, 1], f32, tag="sdv", name="sdv")
                    nc.scalar.activation(sdv[:], var[:], AF.Sqrt)
                    rs = wpool.tile([Fin, 1], f32, tag="rs", name="rs")
                    nc.vector.reciprocal(rs[:], sdv[:])
                    bscale = wpool.tile([Fin, 1], f32, tag="bscale", name="bscale")
                    nc.vector.tensor_tensor(bscale[:], rs[:], bng_sb[l][:], OP.mult)
                    boff = wpool.tile([Fin, 1], f32, tag="boff", name="boff")
                    nc.vector.tensor_tensor(boff[:], mu[:], bscale[:], OP.mult)
                    nc.vector.tensor_tensor(boff[:], bnb_sb[l][:], boff[:], OP.subtract)
                    scale_off = (bscale, boff)

                for t in range(NT):
                    cnt = min(P, NPC - t * P)
                    xt = wpool.tile([P, Fin], f32, tag="xt", name="xt")
                    if cnt < P:
                        nc.vector.memset(xt[:], 0.0)
                    nc.sync.dma_start(out=xt[:cnt, :], in_=src_dram[t * P : t * P + cnt, :Fin])
                    xtp = ppool.tile([Fin, P], f32, tag="xtp", name="xtp", bufs=2)
                    nc.tensor.transpose(out=xtp[:], in_=xt[:], identity=ident_sb[:])
                    xts = wpool.tile([Fin, P], f32, tag="xts", name="xts")
                    if scale_off is not None:
                        nc.vector.tensor_scalar(
                            xts[:], xtp[:], scale_off[0][:], scale_off[1][:], OP.mult, OP.add
                        )
                        nc.scalar.activation(xts[:], xts[:], AF.Relu)
                    else:
                        nc.vector.tensor_copy(xts[:], xtp[:])
                    tt = wpool.tile([P, HC + 8], bf16, tag="tt", name="tt")
                    sd = wpool.tile([P, 8], f32, tag="sd", name="sd")
                    if HC + 16 <= 512:
                        hp = ppool.tile([P, HC + 16], f32, tag="pmm", name="hp", bufs=2)
                        nc.tensor.matmul(out=hp[:], lhsT=xts[:], rhs=ww_sb[l][:], start=True, stop=True)
                        nc.vector.tensor_copy(tt[:], hp[:, : HC + 8])
                        nc.vector.tensor_copy(sd[:], hp[:, HC + 8 : HC + 16])
                    else:  # L4: 528 cols -> split 512 + 16
                        hp = ppool.tile([P, 512], f32, tag="pmm", name="hp", bufs=2)
                        hp2 = ppool.tile([P, 16], f32, tag="pmm2", name="hp2", bufs=1)
                        nc.tensor.matmul(out=hp[:], lhsT=xts[:], rhs=ww_sb[l][:, :512], start=True, stop=True)
                        nc.tensor.matmul(out=hp2[:], lhsT=xts[:], rhs=ww_sb[l][:, 512:], start=True, stop=True)
                        nc.vector.tensor_copy(tt[:, :512], hp[:])
                        nc.vector.tensor_copy(tt[:, 512:520], hp2[:, 0:8])
                        nc.vector.tensor_copy(sd[:], hp2[:, 8:16])
                    nc.sync.dma_start(
                        out=tabL[l][t * P : t * P + cnt, : HC + 8], in_=tt[:cnt, :]
                    )
                    nc.sync.dma_start(out=sdo[t * P : t * P + cnt, :], in_=sd[:cnt, :])

            # ============ edge phase (own dst tiles) ============
            def edge_phase(l, tabA, out_dram):
                cfg = LAYERS[l]
                C, R2 = cfg["C"], cfg["R2"]
                HC = HEADS * C
                do_stat = l in (0, 2)
                if do_stat:
                    accS = wpool.tile([HC, 2], f32, tag="accS", name="accS")
                    nc.vector.memset(accS[:], 0.0)
                for t in range(NT):
                    NCH = NCHT[t]
                    EPT = EPTT[t]
                    cnt = min(P, NPC - t * P)
                    # one-hot O for all chunks of this tile (single DVE op);
                    # pad edges have dstloc=999 -> all-zero column
                    Oall = gpool.tile([P, NCH * P], bf16, tag="Oall", name="Oall")
                    nc.vector.tensor_tensor(
                        Oall[:].rearrange("p (j f) -> p j f", f=P),
                        iotab_sb[:].unsqueeze(1).to_broadcast((P, NCH, P)),
                        dstloc_sb[:, COFF[t] : COFF[t] + NCH].unsqueeze(2).to_broadcast((P, NCH, P)),
                        OP.is_equal,
                    )
                    # own s_dst rows for this tile (local, static offset)
                    sdtf = wpool.tile([P, 8], f32, tag="sdtf", name="sdtf")
                    if cnt < P:
                        nc.vector.memset(sdtf[:], 0.0)
                    nc.sync.dma_start(out=sdtf[:cnt, :], in_=sdo[t * P : t * P + cnt, :])
                    sdtb = wpool.tile([P, 8], bf16, tag="sdtb", name="sdtb")
                    nc.vector.tensor_copy(sdtb[:], sdtf[:])
                    # per-chunk SD = transpose(O_j).T @ sdt  (PE; overlaps gather)
                    psSD = ppool.tile([P, NCH * 8], f32, tag="psSD", name="psSD", bufs=1)
                    for j in range(NCH):
                        psOT = ppool.tile([P, P], bf16, tag="xtp", name="psOT", bufs=2)
                        nc.tensor.transpose(
                            out=psOT[:], in_=Oall[:, j * P : (j + 1) * P], identity=identb_sb[:]
                        )
                        OTs = wpool.tile([P, P], bf16, tag="OTs", name="OTs")
                        nc.scalar.activation(OTs[:], psOT[:], AF.Identity)
                        nc.tensor.matmul(
                            out=psSD[:, j * 8 : (j + 1) * 8], lhsT=OTs[:], rhs=sdtb[:],
                            start=True, stop=True,
                        )
                    # gather source rows [h | s_src] (bf16) from the global table
                    G = gpool.tile([P, NCH * R2], bf16, tag="G", name="G")
                    nc.gpsimd.dma_gather(
                        out_ap=G[:].rearrange("p (j r) -> p j r", r=R2),
                        in_ap=tabA[l][:],
                        idxs_ap=isrc_sb[:, EOFF[t] // 16 : EOFF[t + 1] // 16],
                        num_idxs=EPT,
                        num_idxs_reg=ept_regs[EPT],
                        elem_size=R2,
                        single_packet=False,
                    )
                    G3 = G[:].rearrange("p (j r) -> p j r", r=R2)
                    # batched e = LeakyReLU(s_src + s_dst); EXS = exp(e) (bf16)
                    EB = wpool.tile([P, NCH * 8], f32, tag="EB", name="EB")
                    nc.vector.tensor_copy(
                        EB[:].rearrange("p (j r) -> p j r", r=8), G3[:, :, HC : HC + 8]
                    )
                    nc.vector.tensor_tensor(EB[:], EB[:], psSD[:], OP.add)
                    EB2 = wpool.tile([P, NCH * 8], f32, tag="EB2", name="EB2")
                    nc.vector.tensor_scalar(EB2[:], EB[:], NEG, None, OP.mult)
                    nc.vector.tensor_tensor(EB[:], EB[:], EB2[:], OP.max)
                    EXS = wpool.tile([P, NCH * 8], bf16, tag="EXS", name="EXS")
                    nc.scalar.activation(EXS[:], EB[:], AF.Exp)
                    # per-chunk weighted scatter
                    if HC + 8 <= 512:
                        psA = ppool.tile([P, HC + 8], f32, tag="pmm", name="psA", bufs=2)
                        psB = None
                    else:
                        psA = ppool.tile([P, 512], f32, tag="pmm", name="psA", bufs=2)
                        psB = ppool.tile([P, 8], f32, tag="pmm2", name="psB", bufs=1)
                    for j in range(NCH):
                        GEX = wpool.tile([P, HC + 8], bf16, tag="GEX", name="GEX")
                        nc.vector.tensor_tensor(
                            GEX[:, :HC].rearrange("p (h c) -> p h c", h=HEADS),
                            G3[:, j, :HC].rearrange("p (h c) -> p h c", h=HEADS),
                            EXS[:, j * 8 : (j + 1) * 8].unsqueeze(2).to_broadcast((P, HEADS, C)),
                            OP.mult,
                        )
                        nc.gpsimd.tensor_copy(GEX[:, HC : HC + 8], EXS[:, j * 8 : (j + 1) * 8])
                        if psB is None:
                            nc.tensor.matmul(
                                out=psA[:], lhsT=Oall[:, j * P : (j + 1) * P], rhs=GEX[:],
                                start=(j == 0), stop=(j == NCH - 1),
                            )
                        else:
                            nc.tensor.matmul(
                                out=psA[:], lhsT=Oall[:, j * P : (j + 1) * P], rhs=GEX[:, :512],
                                start=(j == 0), stop=(j == NCH - 1),
                            )
                            nc.tensor.matmul(
                                out=psB[:], lhsT=Oall[:, j * P : (j + 1) * P], rhs=GEX[:, 512:],
                                start=(j == 0), stop=(j == NCH - 1),
                            )
                    den = psA[:, HC : HC + 8] if psB is None else psB[:]
                    rec = wpool.tile([P, 8], f32, tag="rec", name="rec")
                    nc.vector.tensor_scalar(rec[:], den, 1e-16, None, OP.add)
                    nc.vector.reciprocal(rec[:], rec[:])
                    res = wpool.tile([P, HC], f32, tag="res", name="res")
                    nc.vector.tensor_tensor(
                        res[:].rearrange("p (h c) -> p h c", h=HEADS),
                        psA[:, :HC].rearrange("p (h c) -> p h c", h=HEADS),
                        rec[:].unsqueeze(2).to_broadcast((P, HEADS, C)),
                        OP.mult,
                    )
                    if cfg["concat"]:
                        nc.vector.tensor_tensor(res[:], res[:], bb_sb[l][:], OP.add)
                        nc.sync.dma_start(
                            out=out_dram[t * P : t * P + cnt, :], in_=res[:cnt, :]
                        )
                        if do_stat:
                            sq = wpool.tile([P, HC], f32, tag="sq", name="sq")
                            nc.scalar.square(sq[:], res[:])
                            psS1 = ppool.tile([HC, 1], f32, tag="psS", name="psS1", bufs=2)
                            nc.tensor.matmul(
                                out=psS1[:], lhsT=res[:cnt, :], rhs=ones_sb[:cnt, :],
                                start=True, stop=True,
                            )
                            nc.vector.tensor_tensor(accS[:, 0:1], accS[:, 0:1], psS1[:], OP.add)
                            psS2 = ppool.tile([HC, 1], f32, tag="psS", name="psS2", bufs=2)
                            nc.tensor.matmul(
                                out=psS2[:], lhsT=sq[:cnt, :], rhs=ones_sb[:cnt, :],
                                start=True, stop=True,
                            )
                            nc.vector.tensor_tensor(accS[:, 1:2], accS[:, 1:2], psS2[:], OP.add)
                    else:
                        red = wpool.tile([P, C], f32, tag="red", name="red")
                        nc.vector.tensor_reduce(
                            red[:],
                            res[:].rearrange("p (h c) -> p c h", h=HEADS),
                            AX,
                            OP.add,
                        )
                        nc.vector.tensor_scalar(red[:], red[:], 1.0 / HEADS, None, OP.mult)
                        nc.vector.tensor_tensor(red[:], red[:], bb_sb[l][:, :C], OP.add)
                        nc.sync.dma_start(
                            out=out_dram[t * P : t * P + cnt, :], in_=red[:cnt, :]
                        )
                if do_stat:
                    nc.sync.dma_start(out=ostat[l][:], in_=accS[:])

            # ================= full pipeline =================
            for _rep in range(repeat_k):
                tabA, gstat = fresh_shared(_rep)
                # chain reps through out_ext so repeat_k>1 timing can't be
                # dead-code-eliminated (used for timing only; output = model^k)
                srcs = [xin if _rep == 0 else out_ext, own[0], own[1], own[2]]
                outs = [own[0], own[1], own[2], out_ext]
                for l in range(4):
                    node_phase(l, srcs[l], gstat)
                    if single_core:
                        nc.sync.dma_start(out=tabA[l][0:NPC, :], in_=tabL[l][:, :])
                    else:
                        nc.gpsimd.collective_compute(
                            "AllGather",
                            mybir.AluOpType.bypass,
                            replica_groups=[list(range(M))],
                            ins=[tabL[l].opt()],
                            outs=[tabA[l].opt()],
                        )
                    edge_phase(l, tabA, outs[l])
                    if l in (0, 2):
                        if single_core:
                            nc.sync.dma_start(out=gstat[l][:, :], in_=ostat[l][:, :])
                        else:
                            nc.gpsimd.collective_compute(
                                "AllReduce",
                                mybir.AluOpType.add,
                                replica_groups=[list(range(M))],
                                ins=[ostat[l].opt()],
                                outs=[gstat[l].opt()],
                            )
    if not nc.is_finalized():
        nc.finalize()
    return nc


def _pjrt_exec(nc, in_maps, time_reps=0):
    """Mirror of bass2jax.run_bass_via_pjrt multi-core path, holding the jitted
    executable so repeated executions can be wall-timed."""
    import time as _t
    import jax
    from jax.experimental.shard_map import shard_map
    from jax.sharding import Mesh, PartitionSpec
    from concourse import bass2jax as B, mybir as mb

    B.install_neuronx_cc_hook()
    n_cores = len(in_maps)
    partition_name = nc.partition_id_tensor.name if nc.partition_id_tensor else None
    in_names, out_names, out_avals, zero_outs = [], [], [], []
    for alloc in nc.m.functions[0].allocations:
        if not isinstance(alloc, mb.MemoryLocationSet):
            continue
        name = alloc.memorylocations[0].name
        if alloc.kind == "ExternalInput":
            if name != partition_name:
                in_names.append(name)
        elif alloc.kind == "ExternalOutput":
            out_names.append(name)
            shape = tuple(alloc.tensor_shape)
            dtype = mb.dt.np(alloc.dtype)
            out_avals.append(jax.core.ShapedArray(shape, dtype))
            zero_outs.append(np.zeros(shape, dtype))
    n_params = len(in_names)
    n_outs = len(out_avals)
    in_names.extend(out_names)
    if partition_name is not None:
        in_names.append(partition_name)
    donate = tuple(range(n_params, n_params + n_outs))

    def _body(*args):
        operands = list(args)
        if partition_name is not None:
            operands.append(B.partition_id_tensor())
        outs = B._bass_exec_p.bind(
            *operands,
            out_avals=tuple(out_avals),
            in_names=tuple(in_names),
            out_names=tuple(out_names),
            lowering_input_output_aliases=(),
            sim_require_finite=True,
            sim_require_nnan=True,
            nc=nc,
        )
        return tuple(outs)

    devices = jax.devices()[:n_cores]
    mesh = Mesh(np.asarray(devices), ("core",))
    in_specs = (PartitionSpec("core"),) * (n_params + n_outs)
    out_specs = (PartitionSpec("core"),) * len(out_names)
    sharded = jax.jit(
        shard_map(_body, mesh=mesh, in_specs=in_specs, out_specs=out_specs,
                  check_rep=False),
        donate_argnums=donate, keep_unused=True,
    )
    per_core = [[np.asarray(m_[nm]) for nm in in_names[:n_params]] for m_ in in_maps]
    concat_in = [
        np.concatenate([per_core[c][i] for c in range(n_cores)], axis=0)
        for i in range(n_params)
    ]
    from jax.sharding import NamedSharding
    shard = NamedSharding(mesh, PartitionSpec("core"))
    concat_in = [jax.device_put(a, shard) for a in concat_in]
    jax.block_until_ready(concat_in)

    def once():
        cz = [jax.device_put(np.zeros((n_cores * z.shape[0], *z.shape[1:]), z.dtype), shard)
              for z in zero_outs]
        jax.block_until_ready(cz)
        t0 = _t.perf_counter()
        out_arrs = sharded(*concat_in, *cz)
        jax.block_until_ready(out_arrs)
        return _t.perf_counter() - t0, out_arrs

    _, out_arrs = once()  # compile + first run
    times = []
    for _ in range(time_reps):
        dt, out_arrs = once()
        times.append(dt)
    res = [
        {nm: np.asarray(out_arrs[i]).reshape(n_cores, *out_avals[i].shape)[c]
         for i, nm in enumerate(out_names)}
        for c in range(n_cores)
    ]
    return res, (min(times) if times else None)


def _run(inputs, trace=False, time_reps=0, repeat_k=1):
    NCHT, edata = _preprocess(np.asarray(inputs["edge_index"]))
    consts = _host_consts(inputs)
    nc = _build(NCHT, repeat_k=repeat_k)

    x = np.asarray(inputs["x"], dtype=np.float32)
    in_maps = []
    for m in range(M):
        d = dict(consts)
        d.update(edata[m])
        d["xin"] = np.ascontiguousarray(x[m * NPC : (m + 1) * NPC])
        in_maps.append(d)

    if time_reps > 0:
        results, best_s = _pjrt_exec(nc, in_maps, time_reps=time_reps)
    else:
        from concourse.bass_utils import run_bass_kernel_spmd

        res = run_bass_kernel_spmd(nc, in_maps, core_ids=list(range(M)))
        results, best_s = res.results, None
    outs = [np.asarray(results[m]["out"]) for m in range(M)]
    full = np.concatenate(outs, axis=0).astype(np.float32)
    return full, (None if best_s is None else int(best_s * 1e9))


def kernel(**inputs):
    out, _ = _run(inputs, trace=False)
    return out



# revision 22
# speedup vs baseline: 30.1456x; 1.2527x over previous
"""Distributed Bass kernel for a 4-layer GAT autoencoder on 8 TRN2 NeuronCores.

Strategy (per sharding hint): nodes sharded across 8 cores (2500/core);
edges co-located with their destination node's core, sorted by destination;
params replicated.

v3 layout: the node phase is SHARDED — each core transforms only its own
2500 rows (inputs pre-sliced per core host-side, so all node-phase DMA uses
static local offsets), then the compact bf16 per-node tables [h | s_src]
are AllGathered so the edge phase can gather any source row. s_dst stays
local (a [2500,8] buffer). Pad edges carry an out-of-range dstloc (999), so
their one-hot column is all zeros and they contribute nothing to the
scatter or the softmax denominator — no NEGBIG dummy-row machinery.

Edge phase per dst tile: one-hot O for all chunks in one DVE op; a big bf16
dma_gather of source rows; per-edge s_dst via PE transpose(O_j) + an
8-column matmul; batched LeakyReLU/exp; per-chunk h*ex one-hot scatter
matmuls accumulating numerator and denominator in PSUM. BN statistics are
per-core partial sums (closed-group matmuls + SBUF accumulate) + a [128,2]
AllReduce; there is no separate stats pass.
"""

import sys

sys.path.insert(0, "/opt/trn_rl_repo")

import numpy as np

P = 128
M = 8
N = 20000
NPC = N // M  # 2500 nodes per core
NT = (NPC + P - 1) // P  # 20 dst tiles per core
HEADS = 8
NEG = 0.2
BN_EPS = 1e-5
PADDST = 999.0  # out-of-range dst slot for pad edges -> zero one-hot column

# layer configs: Fin, C (per-head out), concat?, bn on input?, bf16 row width R2
LAYERS = [
    dict(Fin=64, C=16, concat=True, bn=False, R2=256),
    dict(Fin=128, C=32, concat=False, bn=True, R2=384),
    dict(Fin=32, C=16, concat=True, bn=False, R2=256),
    dict(Fin=128, C=64, concat=False, bn=True, R2=640),
]
OWNW = [128, 32, 128, 64]  # own[l] row widths


def _wrap16(idx):
    """Host int array -> dma_gather index layout [16, n/16] (idx[s*16+p] at [p,s])."""
    n = idx.shape[0]
    assert n % 16 == 0
    w = np.ascontiguousarray(idx.reshape(n // 16, 16).T).astype(np.int16)
    return np.ascontiguousarray(np.tile(w, (8, 1)))  # replicated for the 8 Q7 cores


def _preprocess(edge_index):
    """Partition + sort edges; per-tile chunk counts; per-core gather indices."""
    src = np.concatenate([np.asarray(edge_index[0]), np.arange(N)]).astype(np.int64)
    dst = np.concatenate([np.asarray(edge_index[1]), np.arange(N)]).astype(np.int64)

    per_core = []
    cnts = np.zeros((M, NT), dtype=np.int64)
    for m in range(M):
        sel = (dst // NPC) == m
        s, d = src[sel], dst[sel]
        dloc = d - NPC * m
        order = np.argsort(dloc, kind="stable")
        s, dloc = s[order], dloc[order]
        tiles = []
        for t in range(NT):
            tsel = (dloc // P) == t
            st, dt_ = s[tsel], dloc[tsel] - t * P
            tiles.append((st, dt_))
            cnts[m, t] = st.shape[0]
        per_core.append(tiles)

    # per-tile chunk count: max over cores, rounded up to a multiple of 4
    # (keeps every tile's idx segment 64B-aligned for the dma_gather ucode)
    nch = np.maximum(((cnts.max(axis=0) + P - 1) // P + 3) // 4 * 4, 4)
    NCHT = [int(v) for v in nch]
    EPTT = [v * P for v in NCHT]
    TOT = int(sum(EPTT))

    data = []
    for m in range(M):
        isrc = np.zeros((TOT,), dtype=np.int64)  # pad edges gather row 0
        dloc_cols = np.full((P, sum(NCHT)), PADDST, dtype=np.float32)
        eoff = 0
        coff = 0
        for t in range(NT):
            st, dt_ = per_core[m][t]
            c = st.shape[0]
            isrc[eoff : eoff + c] = st
            dl = np.full((EPTT[t],), PADDST, dtype=np.float64)
            dl[:c] = dt_
            # column coff+j, row p  = edge (t, j*128+p)
            dloc_cols[:, coff : coff + NCHT[t]] = dl.reshape(NCHT[t], P).T
            eoff += EPTT[t]
            coff += NCHT[t]
        # host-precomputed transposed one-hot: OT[n, c*128+q] = 1 iff edge
        # (chunk col c, slot q) targets local node n  (pad cols all-zero).
        # shipped fp8 (0/1 exact) to halve the input transfer; converted to
        # bf16 once on-chip.
        import ml_dtypes

        TOTC = sum(NCHT)
        OT = np.zeros((P, TOTC * P), dtype=ml_dtypes.float8_e4m3)
        d_ = dloc_cols.astype(np.int64)  # [q, c]
        q_idx, c_idx = np.nonzero(d_ < P)
        OT[d_[q_idx, c_idx], c_idx * P + q_idx] = 1
        data.append(
            dict(idx_src=_wrap16(isrc), dstloc=np.ascontiguousarray(dloc_cols),
                 otall=OT)
        )
    return NCHT, data


def _host_consts(inputs):
    """Fused weights + broadcast biases + misc consts."""
    f32 = np.float32
    c = {}
    c["iotab"] = np.tile(np.arange(P, dtype=f32)[None, :], (P, 1))
    c["ident"] = np.eye(P, dtype=f32)
    c["ones"] = np.ones((P, 1), dtype=f32)

    def fuse(W, a_s, a_d):
        # WW = [W | W@blockdiag(a_src) | W@blockdiag(a_dst)]  -> [Fin, HC+16]
        H, C_ = a_s.shape
        Ws = np.einsum("fhc,hc->fh", W.reshape(-1, H, C_), a_s)
        Wd = np.einsum("fhc,hc->fh", W.reshape(-1, H, C_), a_d)
        return np.concatenate([W, Ws, Wd], axis=1).astype(f32)

    c["ww1"] = fuse(inputs["We1"], inputs["as_e1"], inputs["ad_e1"])
    c["ww2"] = fuse(inputs["We2"], inputs["as_e2"], inputs["ad_e2"])
    c["ww3"] = fuse(inputs["Wd1"], inputs["as_d1"], inputs["ad_d1"])
    c["ww4"] = fuse(inputs["Wd2"], inputs["as_d2"], inputs["ad_d2"])
    c["bb1"] = np.tile(inputs["b_e1"][None, :], (P, 1)).astype(f32)  # [128,128]
    c["bb2"] = np.tile(inputs["b_e2"][None, :], (P, 1)).astype(f32)  # [128,32]
    c["bb3"] = np.tile(inputs["b_d1"][None, :], (P, 1)).astype(f32)  # [128,128]
    c["bb4"] = np.tile(inputs["b_d2"][None, :], (P, 1)).astype(f32)  # [128,64]
    c["bn1g"] = inputs["bn1_g"].astype(f32).reshape(-1, 1)  # [128,1]
    c["bn1b"] = inputs["bn1_b"].astype(f32).reshape(-1, 1)
    c["bn2g"] = inputs["bn2_g"].astype(f32).reshape(-1, 1)
    c["bn2b"] = inputs["bn2_b"].astype(f32).reshape(-1, 1)
    return c


def _build(NCHT, repeat_k=1, single_core=False, no_coll=False):
    from concourse import bacc, bass, mybir, tile

    f32 = mybir.dt.float32
    bf16 = mybir.dt.bfloat16
    i16 = mybir.dt.int16
    nc = bacc.Bacc(
        "TRN2",
        target_bir_lowering=False,
        debug=False,
        enable_asserts=False,
        num_devices=1 if single_core else M,
    )
    local_coll = single_core or no_coll  # replace collectives w/ local copies

    EPTT = [v * P for v in NCHT]
    TOT = sum(EPTT)
    TOTC = sum(NCHT)
    EOFF = np.concatenate([[0], np.cumsum(EPTT)]).astype(int)
    COFF = np.concatenate([[0], np.cumsum(NCHT)]).astype(int)

    def din(name, shape, dtype=f32):
        return nc.dram_tensor(name, list(shape), dtype, kind="ExternalInput")

    fp8 = mybir.dt.float8e4
    xin = din("xin", (NPC, 64))  # per-core slice of x
    idx_src = din("idx_src", (128, TOT // 16), i16)
    dstloc = din("dstloc", (P, TOTC))
    otall = din("otall", (P, TOT), fp8)
    iotab = din("iotab", (P, P))
    ident = din("ident", (P, P))
    ones = din("ones", (P, 1))
    ww = [din(f"ww{l + 1}", (LAYERS[l]["Fin"], HEADS * LAYERS[l]["C"] + 16)) for l in range(4)]
    bb = [
        din("bb1", (P, 128)),
        din("bb2", (P, 32)),
        din("bb3", (P, 128)),
        din("bb4", (P, 64)),
    ]
    bng = [None, din("bn1g", (128, 1)), None, din("bn2g", (128, 1))]
    bnb = [None, din("bn1b", (128, 1)), None, din("bn2b", (128, 1))]
    out_ext = nc.dram_tensor("out", [NPC, 64], f32, kind="ExternalOutput")

    with tile.TileContext(nc) as tc:
        with (
            tc.tile_pool(name="dram", bufs=1, space="DRAM") as dram,
            tc.tile_pool(name="const", bufs=1) as cpool,
            tc.tile_pool(name="work", bufs=3) as wpool,
            tc.tile_pool(name="gath", bufs=2) as gpool,
            tc.tile_pool(name="psum", bufs=2, space="PSUM") as ppool,
        ):
            # ---- internal DRAM (local) ----
            tabL = [
                dram.tile([NPC, LAYERS[l]["R2"]], bf16, tag=f"tabL{l}", name=f"tabL{l}")
                for l in range(4)
            ]
            sdo = dram.tile([NPC, 8], f32, tag="sdo", name="sdo")
            own = [
                dram.tile([NPC, OWNW[l]], f32, tag=f"own{l}", name=f"own{l}")
                for l in range(3)
            ]
            ostat = [
                dram.tile([128, 2], f32, tag="ostat0", name="ostat0"),
                None,
                dram.tile([128, 2], f32, tag="ostat2", name="ostat2"),
            ]

            def fresh_shared(rep):
                sfx = "" if rep == 0 else f"r{rep}"
                aspace = {} if local_coll else dict(addr_space="Shared")
                tabA = [
                    dram.tile([N, LAYERS[l]["R2"]], bf16, tag=f"tabA{l}{sfx}",
                              name=f"tabA{l}{sfx}", **aspace)
                    for l in range(4)
                ]
                gstat = [
                    dram.tile([128, 2], f32, tag=f"gs0{sfx}", name=f"gs0{sfx}", **aspace),
                    None,
                    dram.tile([128, 2], f32, tag=f"gs2{sfx}", name=f"gs2{sfx}", **aspace),
                ]
                return tabA, gstat

            # ---- consts to SBUF ----
            def load_const(ap, shape, dtype=f32, tag=None):
                t = cpool.tile(list(shape), dtype, tag=tag, name=tag)
                nc.sync.dma_start(out=t[:], in_=ap[:])
                return t

            iotaf_sb = load_const(iotab, (P, P), tag="iotaf")
            ident_sb = load_const(ident, (P, P), tag="ident")
            ones_sb = load_const(ones, (P, 1), tag="ones")
            isrc_sb = load_const(idx_src, (128, TOT // 16), i16, tag="isrc")
            dstloc_sb = load_const(dstloc, (P, TOTC), tag="dstloc")
            ww_sb = [
                load_const(ww[l], (LAYERS[l]["Fin"], HEADS * LAYERS[l]["C"] + 16), tag=f"ww{l}")
                for l in range(4)
            ]
            bb_sb = [
                load_const(bb[0], (P, 128), tag="bb0"),
                load_const(bb[1], (P, 32), tag="bb1"),
                load_const(bb[2], (P, 128), tag="bb2"),
                load_const(bb[3], (P, 64), tag="bb3"),
            ]
            bng_sb = [None, load_const(bng[1], (128, 1), tag="bng1"), None, load_const(bng[3], (128, 1), tag="bng3")]
            bnb_sb = [None, load_const(bnb[1], (128, 1), tag="bnb1"), None, load_const(bnb[3], (128, 1), tag="bnb3")]

            AX = mybir.AxisListType.X
            OP = mybir.AluOpType
            AF = mybir.ActivationFunctionType

            iotab_sb = cpool.tile([P, P], bf16, tag="iotabf", name="iotabf")
            nc.vector.tensor_copy(iotab_sb[:], iotaf_sb[:])

            # stream the fp8 transposed one-hot into a persistent bf16 buffer
            # (small staging tile; conversion on the idle Act engine)
            otall_sb = cpool.tile([P, TOT], bf16, tag="otall", name="otall")
            OTCH = TOT // 8
            for ci in range(8):
                stg = wpool.tile([P, OTCH], fp8, tag="otstage", name="otstage", bufs=2)
                nc.sync.dma_start(out=stg[:], in_=otall[:, ci * OTCH : (ci + 1) * OTCH])
                nc.scalar.activation(
                    otall_sb[:, ci * OTCH : (ci + 1) * OTCH], stg[:], AF.Identity
                )

            # registers holding num_idxs values for dma_gather
            ept_regs = {}
            for v in sorted(set(EPTT)):
                r = nc.alloc_registers(name=f"ept{v}")
                nc.regs_mov(r, v)
                ept_regs[v] = nc.snap(r, donate=False)

            # ============ node phase (own 2500 rows only) ============
            def node_phase(l, src_dram, gstat):
                cfg = LAYERS[l]
                Fin, C, R2 = cfg["Fin"], cfg["C"], cfg["R2"]
                HC = HEADS * C
                scale_off = None
                if cfg["bn"]:
                    sg = wpool.tile([128, 2], f32, tag="sg", name="sg")
                    nc.sync.dma_start(out=sg[:], in_=gstat[l - 1][:])
                    mu = wpool.tile([Fin, 1], f32, tag="mu", name="mu")
                    nc.vector.tensor_scalar(mu[:], sg[:, 0:1], 1.0 / N, None, OP.mult)
                    msq = wpool.tile([Fin, 1], f32, tag="msq", name="msq")
                    nc.vector.tensor_scalar(msq[:], sg[:, 1:2], 1.0 / N, None, OP.mult)
                    var = wpool.tile([Fin, 1], f32, tag="var", name="var")
                    nc.vector.tensor_tensor(var[:], mu[:], mu[:], OP.mult)
                    nc.vector.tensor_tensor(var[:], msq[:], var[:], OP.subtract)
                    nc.vector.tensor_scalar(var[:], var[:], BN_EPS, None, OP.add)
                    sdv = wpool.tile([Fin, 1], f32, tag="sdv", name="sdv")
                    nc.scalar.activation(sdv[:], var[:], AF.Sqrt)
                    rs = wpool.tile([Fin, 1], f32, tag="rs", name="rs")
                    nc.vector.reciprocal(rs[:], sdv[:])
                    bscale = wpool.tile([Fin, 1], f32, tag="bscale", name="bscale")
                    nc.vector.tensor_tensor(bscale[:], rs[:], bng_sb[l][:], OP.mult)
                    boff = wpool.tile([Fin, 1], f32, tag="boff", name="boff")
                    nc.vector.tensor_tensor(boff[:], mu[:], bscale[:], OP.mult)
                    nc.vector.tensor_tensor(boff[:], bnb_sb[l][:], boff[:], OP.subtract)
                    scale_off = (bscale, boff)

                for t in range(NT):
                    cnt = min(P, NPC - t * P)
                    xt = wpool.tile([P, Fin], f32, tag="xt", name="xt")
                    if cnt < P:
                        nc.vector.memset(xt[:], 0.0)
                    nc.sync.dma_start(out=xt[:cnt, :], in_=src_dram[t * P : t * P + cnt, :Fin])
                    xtp = ppool.tile([Fin, P], f32, tag="xtp", name="xtp", bufs=2)
                    nc.tensor.transpose(out=xtp[:], in_=xt[:], identity=ident_sb[:])
                    xts = wpool.tile([Fin, P], f32, tag="xts", name="xts")
                    if scale_off is not None:
                        # fused BN affine + relu on the Act engine
                        nc.scalar.activation(
                            xts[:], xtp[:], AF.Relu,
                            bias=scale_off[1][:], scale=scale_off[0][:],
                        )
                    else:
                        nc.scalar.activation(xts[:], xtp[:], AF.Identity)
                    tt = wpool.tile([P, HC + 8], bf16, tag="tt", name="tt")
                    sd = wpool.tile([P, 8], f32, tag="sd", name="sd")
                    if HC + 16 <= 512:
                        hp = ppool.tile([P, HC + 16], f32, tag="pmm", name="hp", bufs=2)
                        nc.tensor.matmul(out=hp[:], lhsT=xts[:], rhs=ww_sb[l][:], start=True, stop=True)
                        nc.vector.tensor_copy(tt[:], hp[:, : HC + 8])
                        nc.vector.tensor_copy(sd[:], hp[:, HC + 8 : HC + 16])
                    else:  # L4: 528 cols -> split 512 + 16
                        hp = ppool.tile([P, 512], f32, tag="pmm", name="hp", bufs=2)
                        hp2 = ppool.tile([P, 16], f32, tag="pmm2", name="hp2", bufs=1)
                        nc.tensor.matmul(out=hp[:], lhsT=xts[:], rhs=ww_sb[l][:, :512], start=True, stop=True)
                        nc.tensor.matmul(out=hp2[:], lhsT=xts[:], rhs=ww_sb[l][:, 512:], start=True, stop=True)
                        nc.vector.tensor_copy(tt[:, :512], hp[:])
                        nc.vector.tensor_copy(tt[:, 512:520], hp2[:, 0:8])
                        nc.vector.tensor_copy(sd[:], hp2[:, 8:16])
                    nc.sync.dma_start(
                        out=tabL[l][t * P : t * P + cnt, : HC + 8], in_=tt[:cnt, :]
                    )
                    nc.sync.dma_start(out=sdo[t * P : t * P + cnt, :], in_=sd[:cnt, :])

            # ============ edge phase (own dst tiles) ============
            def edge_phase(l, tabA, out_dram):
                cfg = LAYERS[l]
                C, R2 = cfg["C"], cfg["R2"]
                HC = HEADS * C
                do_stat = l in (0, 2)
                if do_stat:
                    accS = wpool.tile([HC, 2], f32, tag="accS", name="accS")
                    nc.vector.memset(accS[:], 0.0)
                for t in range(NT):
                    NCH = NCHT[t]
                    EPT = EPTT[t]
                    cnt = min(P, NPC - t * P)
                    # one-hot O for all chunks of this tile (single DVE op);
                    # pad edges have dstloc=999 -> all-zero column
                    Oall = gpool.tile([P, NCH * P], bf16, tag="Oall", name="Oall")
                    nc.vector.tensor_tensor(
                        Oall[:].rearrange("p (j f) -> p j f", f=P),
                        iotab_sb[:].unsqueeze(1).to_broadcast((P, NCH, P)),
                        dstloc_sb[:, COFF[t] : COFF[t] + NCH].unsqueeze(2).to_broadcast((P, NCH, P)),
                        OP.is_equal,
                    )
                    # own s_dst rows for this tile (local, static offset)
                    sdtf = wpool.tile([P, 8], f32, tag="sdtf", name="sdtf")
                    if cnt < P:
                        nc.vector.memset(sdtf[:], 0.0)
                    nc.sync.dma_start(out=sdtf[:cnt, :], in_=sdo[t * P : t * P + cnt, :])
                    sdtb = wpool.tile([P, 8], bf16, tag="sdtb", name="sdtb")
                    nc.vector.tensor_copy(sdtb[:], sdtf[:])
                    # per-chunk SD[e,h] = s_dst[dst(e),h] via matmul with the
                    # host-precomputed transposed one-hot (no PE transposes)
                    psSD = ppool.tile([P, NCH * 8], f32, tag="psSD", name="psSD", bufs=1)
                    for j in range(NCH):
                        blk = (COFF[t] + j) * P
                        nc.tensor.matmul(
                            out=psSD[:, j * 8 : (j + 1) * 8],
                            lhsT=otall_sb[:, blk : blk + P], rhs=sdtb[:],
                            start=True, stop=True,
                        )
                    # gather source rows [h | s_src] (bf16) from the global table
                    G = gpool.tile([P, NCH * R2], bf16, tag="G", name="G")
                    nc.gpsimd.dma_gather(
                        out_ap=G[:].rearrange("p (j r) -> p j r", r=R2),
                        in_ap=tabA[l][:],
                        idxs_ap=isrc_sb[:, EOFF[t] // 16 : EOFF[t + 1] // 16],
                        num_idxs=EPT,
                        num_idxs_reg=ept_regs[EPT],
                        elem_size=R2,
                        single_packet=False,
                    )
                    G3 = G[:].rearrange("p (j r) -> p j r", r=R2)
                    # batched e = LeakyReLU(s_src + s_dst); EXS = exp(e) (bf16)
                    EB = wpool.tile([P, NCH * 8], f32, tag="EB", name="EB")
                    nc.vector.tensor_tensor(
                        EB[:].rearrange("p (j r) -> p j r", r=8),
                        G3[:, :, HC : HC + 8],
                        psSD[:].rearrange("p (j r) -> p j r", r=8),
                        OP.add,
                    )
                    EB2 = wpool.tile([P, NCH * 8], f32, tag="EB2", name="EB2")
                    nc.vector.tensor_scalar(EB2[:], EB[:], NEG, None, OP.mult)
                    nc.vector.tensor_tensor(EB[:], EB[:], EB2[:], OP.max)
                    EXS = wpool.tile([P, NCH * 8], bf16, tag="EXS", name="EXS")
                    nc.scalar.activation(EXS[:], EB[:], AF.Exp)
                    # h *= exp(e) in place on G (one 4D DVE op for all chunks)
                    nc.vector.tensor_tensor(
                        G3[:, :, :HC].rearrange("p j (h c) -> p j h c", c=C),
                        G3[:, :, :HC].rearrange("p j (h c) -> p j h c", c=C),
                        EXS[:].rearrange("p (j h) -> p j h", h=8)
                        .unsqueeze(3).to_broadcast((P, NCH, 8, C)),
                        OP.mult,
                    )
                    # overwrite the s_src columns with exp(e) (denominator)
                    nc.scalar.activation(
                        G3[:, :, HC : HC + 8],
                        EXS[:].rearrange("p (j r) -> p j r", r=8),
                        AF.Identity,
                    )
                    # per-chunk weighted scatter straight out of G
                    if HC + 8 <= 512:
                        psA = ppool.tile([P, HC + 8], f32, tag="pmm", name="psA", bufs=2)
                        psB = None
                    else:
                        psA = ppool.tile([P, 512], f32, tag="pmm", name="psA", bufs=2)
                        psB = ppool.tile([P, 8], f32, tag="pmm2", name="psB", bufs=1)
                    for j in range(NCH):
                        if psB is None:
                            nc.tensor.matmul(
                                out=psA[:], lhsT=Oall[:, j * P : (j + 1) * P],
                                rhs=G3[:, j, : HC + 8],
                                start=(j == 0), stop=(j == NCH - 1),
                            )
                        else:
                            nc.tensor.matmul(
                                out=psA[:], lhsT=Oall[:, j * P : (j + 1) * P],
                                rhs=G3[:, j, :512],
                                start=(j == 0), stop=(j == NCH - 1),
                            )
                            nc.tensor.matmul(
                                out=psB[:], lhsT=Oall[:, j * P : (j + 1) * P],
                                rhs=G3[:, j, 512 : HC + 8],
                                start=(j == 0), stop=(j == NCH - 1),
                            )
                    den = psA[:, HC : HC + 8] if psB is None else psB[:]
                    rec = wpool.tile([P, 8], f32, tag="rec", name="rec")
                    nc.vector.tensor_scalar(rec[:], den, 1e-16, None, OP.add)
                    nc.vector.reciprocal(rec[:], rec[:])
                    res = wpool.tile([P, HC], f32, tag="res", name="res")
                    nc.vector.tensor_tensor(
                        res[:].rearrange("p (h c) -> p h c", h=HEADS),
                        psA[:, :HC].rearrange("p (h c) -> p h c", h=HEADS),
                        rec[:].unsqueeze(2).to_broadcast((P, HEADS, C)),
                        OP.mult,
                    )
                    if cfg["concat"]:
                        nc.vector.tensor_tensor(res[:], res[:], bb_sb[l][:], OP.add)
                        nc.sync.dma_start(
                            out=out_dram[t * P : t * P + cnt, :], in_=res[:cnt, :]
                        )
                        if do_stat:
                            sq = wpool.tile([P, HC], f32, tag="sq", name="sq")
                            nc.scalar.square(sq[:], res[:])
                            psS1 = ppool.tile([HC, 1], f32, tag="psS", name="psS1", bufs=2)
                            nc.tensor.matmul(
                                out=psS1[:], lhsT=res[:cnt, :], rhs=ones_sb[:cnt, :],
                                start=True, stop=True,
                            )
                            nc.vector.tensor_tensor(accS[:, 0:1], accS[:, 0:1], psS1[:], OP.add)
                            psS2 = ppool.tile([HC, 1], f32, tag="psS", name="psS2", bufs=2)
                            nc.tensor.matmul(
                                out=psS2[:], lhsT=sq[:cnt, :], rhs=ones_sb[:cnt, :],
                                start=True, stop=True,
                            )
                            nc.vector.tensor_tensor(accS[:, 1:2], accS[:, 1:2], psS2[:], OP.add)
                    else:
                        red = wpool.tile([P, C], f32, tag="red", name="red")
                        nc.vector.tensor_reduce(
                            red[:],
                            res[:].rearrange("p (h c) -> p c h", h=HEADS),
                            AX,
                            OP.add,
                        )
                        nc.vector.tensor_scalar(red[:], red[:], 1.0 / HEADS, None, OP.mult)
                        nc.vector.tensor_tensor(red[:], red[:], bb_sb[l][:, :C], OP.add)
                        nc.sync.dma_start(
                            out=out_dram[t * P : t * P + cnt, :], in_=red[:cnt, :]
                        )
                if do_stat:
                    nc.sync.dma_start(out=ostat[l][:], in_=accS[:])

            # ================= full pipeline =================
            for _rep in range(repeat_k):
                tabA, gstat = fresh_shared(_rep)
                # chain reps through out_ext so repeat_k>1 timing can't be
                # dead-code-eliminated (used for timing only; output = model^k)
                srcs = [xin if _rep == 0 else out_ext, own[0], own[1], own[2]]
                outs = [own[0], own[1], own[2], out_ext]
                for l in range(4):
                    node_phase(l, srcs[l], gstat)
                    if local_coll:
                        nc.sync.dma_start(out=tabA[l][0:NPC, :], in_=tabL[l][:, :])
                    else:
                        nc.gpsimd.collective_compute(
                            "AllGather",
                            mybir.AluOpType.bypass,
                            replica_groups=[list(range(M))],
                            ins=[tabL[l].opt()],
                            outs=[tabA[l].opt()],
                        )
                    edge_phase(l, tabA, outs[l])
                    if l in (0, 2):
                        if local_coll:
                            nc.sync.dma_start(out=gstat[l][:, :], in_=ostat[l][:, :])
                        else:
                            nc.gpsimd.collective_compute(
                                "AllReduce",
                                mybir.AluOpType.add,
                                replica_groups=[list(range(M))],
                                ins=[ostat[l].opt()],
                                outs=[gstat[l].opt()],
                            )
    if not nc.is_finalized():
        nc.finalize()
    return nc


def _pjrt_exec(nc, in_maps, time_reps=0):
    """Mirror of bass2jax.run_bass_via_pjrt multi-core path, holding the jitted
    executable so repeated executions can be wall-timed."""
    import time as _t
    import jax
    from jax.experimental.shard_map import shard_map
    from jax.sharding import Mesh, PartitionSpec
    from concourse import bass2jax as B, mybir as mb

    B.install_neuronx_cc_hook()
    n_cores = len(in_maps)
    partition_name = nc.partition_id_tensor.name if nc.partition_id_tensor else None
    in_names, out_names, out_avals, zero_outs = [], [], [], []
    for alloc in nc.m.functions[0].allocations:
        if not isinstance(alloc, mb.MemoryLocationSet):
            continue
        name = alloc.memorylocations[0].name
        if alloc.kind == "ExternalInput":
            if name != partition_name:
                in_names.append(name)
        elif alloc.kind == "ExternalOutput":
            out_names.append(name)
            shape = tuple(alloc.tensor_shape)
            dtype = mb.dt.np(alloc.dtype)
            out_avals.append(jax.core.ShapedArray(shape, dtype))
            zero_outs.append(np.zeros(shape, dtype))
    n_params = len(in_names)
    n_outs = len(out_avals)
    in_names.extend(out_names)
    if partition_name is not None:
        in_names.append(partition_name)
    donate = tuple(range(n_params, n_params + n_outs))

    def _body(*args):
        operands = list(args)
        if partition_name is not None:
            operands.append(B.partition_id_tensor())
        outs = B._bass_exec_p.bind(
            *operands,
            out_avals=tuple(out_avals),
            in_names=tuple(in_names),
            out_names=tuple(out_names),
            lowering_input_output_aliases=(),
            sim_require_finite=True,
            sim_require_nnan=True,
            nc=nc,
        )
        return tuple(outs)

    devices = jax.devices()[:n_cores]
    mesh = Mesh(np.asarray(devices), ("core",))
    in_specs = (PartitionSpec("core"),) * (n_params + n_outs)
    out_specs = (PartitionSpec("core"),) * len(out_names)
    sharded = jax.jit(
        shard_map(_body, mesh=mesh, in_specs=in_specs, out_specs=out_specs,
                  check_rep=False),
        donate_argnums=donate, keep_unused=True,
    )
    per_core = [[np.asarray(m_[nm]) for nm in in_names[:n_params]] for m_ in in_maps]
    concat_in = [
        np.concatenate([per_core[c][i] for c in range(n_cores)], axis=0)
        for i in range(n_params)
    ]
    from jax.sharding import NamedSharding
    shard = NamedSharding(mesh, PartitionSpec("core"))
    concat_in = [jax.device_put(a, shard) for a in concat_in]
    jax.block_until_ready(concat_in)

    def once():
        cz = [jax.device_put(np.zeros((n_cores * z.shape[0], *z.shape[1:]), z.dtype), shard)
              for z in zero_outs]
        jax.block_until_ready(cz)
        t0 = _t.perf_counter()
        out_arrs = sharded(*concat_in, *cz)
        jax.block_until_ready(out_arrs)
        return _t.perf_counter() - t0, out_arrs

    _, out_arrs = once()  # compile + first run
    times = []
    for _ in range(time_reps):
        dt, out_arrs = once()
        times.append(dt)
    res = [
        {nm: np.asarray(out_arrs[i]).reshape(n_cores, *out_avals[i].shape)[c]
         for i, nm in enumerate(out_names)}
        for c in range(n_cores)
    ]
    # free device buffers so back-to-back _run calls don't accumulate HBM use
    import gc

    for a in list(concat_in) + list(out_arrs):
        try:
            a.delete()
        except Exception:
            pass
    del concat_in, out_arrs
    gc.collect()
    return res, (min(times) if times else None)


def _run(inputs, trace=False, time_reps=0, repeat_k=1, no_coll=False):
    NCHT, edata = _preprocess(np.asarray(inputs["edge_index"]))
    consts = _host_consts(inputs)
    nc = _build(NCHT, repeat_k=repeat_k, no_coll=no_coll)

    x = np.asarray(inputs["x"], dtype=np.float32)
    in_maps = []
    for m in range(M):
        d = dict(consts)
        d.update(edata[m])
        d["xin"] = np.ascontiguousarray(x[m * NPC : (m + 1) * NPC])
        in_maps.append(d)

    if time_reps > 0:
        results, best_s = _pjrt_exec(nc, in_maps, time_reps=time_reps)
    else:
        from concourse.bass_utils import run_bass_kernel_spmd

        res = run_bass_kernel_spmd(nc, in_maps, core_ids=list(range(M)))
        results, best_s = res.results, None
    outs = [np.asarray(results[m]["out"]) for m in range(M)]
    full = np.concatenate(outs, axis=0).astype(np.float32)
    return full, (None if best_s is None else int(best_s * 1e9))


def kernel(**inputs):
    out, _ = _run(inputs, trace=False)
    return out



# revision 37
# speedup vs baseline: 33.8281x; 1.1222x over previous
"""Distributed Bass kernel for a 4-layer GAT autoencoder on 8 TRN2 NeuronCores.

Strategy (per sharding hint): nodes sharded across 8 cores (2500/core);
edges co-located with their destination node's core, sorted by destination;
params replicated.

v3 layout: the node phase is SHARDED — each core transforms only its own
2500 rows (inputs pre-sliced per core host-side, so all node-phase DMA uses
static local offsets), then the compact bf16 per-node tables [h | s_src]
are AllGathered so the edge phase can gather any source row. s_dst stays
local (a [2500,8] buffer). Pad edges carry an out-of-range dstloc (999), so
their one-hot column is all zeros and they contribute nothing to the
scatter or the softmax denominator — no NEGBIG dummy-row machinery.

Edge phase per dst tile: one-hot O for all chunks in one DVE op; a big bf16
dma_gather of source rows; per-edge s_dst via PE transpose(O_j) + an
8-column matmul; batched LeakyReLU/exp; per-chunk h*ex one-hot scatter
matmuls accumulating numerator and denominator in PSUM. BN statistics are
per-core partial sums (closed-group matmuls + SBUF accumulate) + a [128,2]
AllReduce; there is no separate stats pass.
"""

import sys

sys.path.insert(0, "/opt/trn_rl_repo")

import numpy as np

P = 128
M = 8
N = 20000
NPC = N // M  # 2500 nodes per core
NT = (NPC + P - 1) // P  # 20 dst tiles per core
HEADS = 8
NEG = 0.2
BN_EPS = 1e-5
PADDST = 999.0  # out-of-range dst slot for pad edges -> zero one-hot column

# layer configs: Fin, C (per-head out), concat?, bn on input?, bf16 row width R2
LAYERS = [
    dict(Fin=64, C=16, concat=True, bn=False, R2=256),
    dict(Fin=128, C=32, concat=False, bn=True, R2=384),
    dict(Fin=32, C=16, concat=True, bn=False, R2=256),
    dict(Fin=128, C=64, concat=False, bn=True, R2=640),
]
OWNW = [128, 32, 128, 64]  # own[l] row widths


def _wrap16(idx):
    """Host int array -> dma_gather index layout [16, n/16] (idx[s*16+p] at [p,s])."""
    n = idx.shape[0]
    assert n % 16 == 0
    w = np.ascontiguousarray(idx.reshape(n // 16, 16).T).astype(np.int16)
    return np.ascontiguousarray(np.tile(w, (8, 1)))  # replicated for the 8 Q7 cores


def _preprocess(edge_index):
    """Partition + sort edges; per-tile chunk counts; per-core gather indices."""
    src = np.concatenate([np.asarray(edge_index[0]), np.arange(N)]).astype(np.int64)
    dst = np.concatenate([np.asarray(edge_index[1]), np.arange(N)]).astype(np.int64)

    per_core = []
    cnts = np.zeros((M, NT), dtype=np.int64)
    for m in range(M):
        sel = (dst // NPC) == m
        s, d = src[sel], dst[sel]
        dloc = d - NPC * m
        order = np.argsort(dloc, kind="stable")
        s, dloc = s[order], dloc[order]
        tiles = []
        for t in range(NT):
            tsel = (dloc // P) == t
            st, dt_ = s[tsel], dloc[tsel] - t * P
            tiles.append((st, dt_))
            cnts[m, t] = st.shape[0]
        per_core.append(tiles)

    # per-tile chunk count: max over cores, rounded up to a multiple of 4
    # (keeps every tile's idx segment 64B-aligned for the dma_gather ucode)
    nch = np.maximum(((cnts.max(axis=0) + P - 1) // P + 3) // 4 * 4, 4)
    NCHT = [int(v) for v in nch]
    EPTT = [v * P for v in NCHT]
    TOT = int(sum(EPTT))

    # effective chunk count per tile: only chunks holding real edges (the
    # x4-rounded layout keeps its offsets; trailing all-pad chunks are simply
    # never gathered or processed). All gathered slots use idx>=0 (pads gather
    # row 0) -- negative-index skipping produced garbage on HW.
    NCHE = [min((int(cnts.max(axis=0)[t]) + P - 1) // P, NCHT[t]) for t in range(NT)]
    VALT = [NCHE[t] * P for t in range(NT)]

    data = []
    for m in range(M):
        isrc = np.zeros((TOT,), dtype=np.int64)  # pad edges gather row 0
        dloc_cols = np.full((P, sum(NCHT)), PADDST, dtype=np.float32)
        eoff = 0
        coff = 0
        for t in range(NT):
            st, dt_ = per_core[m][t]
            c = st.shape[0]
            isrc[eoff : eoff + c] = st
            dl = np.full((EPTT[t],), PADDST, dtype=np.float64)
            dl[:c] = dt_
            # column coff+j, row p  = edge (t, j*128+p)
            dloc_cols[:, coff : coff + NCHT[t]] = dl.reshape(NCHT[t], P).T
            eoff += EPTT[t]
            coff += NCHT[t]
        # host-precomputed transposed one-hot: OT[n, c*128+q] = 1 iff edge
        # (chunk col c, slot q) targets local node n  (pad cols all-zero).
        # shipped fp8 (0/1 exact) to halve the input transfer; converted to
        # bf16 once on-chip.
        import ml_dtypes

        TOTC = sum(NCHT)
        OT = np.zeros((P, TOTC * P), dtype=ml_dtypes.float8_e4m3)
        d_ = dloc_cols.astype(np.int64)  # [q, c]
        q_idx, c_idx = np.nonzero(d_ < P)
        OT[d_[q_idx, c_idx], c_idx * P + q_idx] = 1
        data.append(
            dict(idx_src=_wrap16(isrc), dstloc=np.ascontiguousarray(dloc_cols),
                 otall=OT)
        )
    return NCHT, VALT, NCHE, data


def _host_consts(inputs):
    """Fused weights + broadcast biases + misc consts."""
    f32 = np.float32
    c = {}
    c["iotab"] = np.tile(np.arange(P, dtype=f32)[None, :], (P, 1))
    c["ident"] = np.eye(P, dtype=f32)
    c["ones"] = np.ones((P, 1), dtype=f32)

    def fuse(W, a_s, a_d):
        # WW = [W | W@blockdiag(a_src) | W@blockdiag(a_dst)]  -> [Fin, HC+16]
        H, C_ = a_s.shape
        Ws = np.einsum("fhc,hc->fh", W.reshape(-1, H, C_), a_s)
        Wd = np.einsum("fhc,hc->fh", W.reshape(-1, H, C_), a_d)
        return np.concatenate([W, Ws, Wd], axis=1).astype(f32)

    c["ww1"] = fuse(inputs["We1"], inputs["as_e1"], inputs["ad_e1"])
    c["ww2"] = fuse(inputs["We2"], inputs["as_e2"], inputs["ad_e2"])
    c["ww3"] = fuse(inputs["Wd1"], inputs["as_d1"], inputs["ad_d1"])
    c["ww4"] = fuse(inputs["Wd2"], inputs["as_d2"], inputs["ad_d2"])
    c["bb1"] = np.tile(inputs["b_e1"][None, :], (P, 1)).astype(f32)  # [128,128]
    c["bb2"] = np.tile(inputs["b_e2"][None, :], (P, 1)).astype(f32)  # [128,32]
    c["bb3"] = np.tile(inputs["b_d1"][None, :], (P, 1)).astype(f32)  # [128,128]
    c["bb4"] = np.tile(inputs["b_d2"][None, :], (P, 1)).astype(f32)  # [128,64]
    c["bn1g"] = inputs["bn1_g"].astype(f32).reshape(-1, 1)  # [128,1]
    c["bn1b"] = inputs["bn1_b"].astype(f32).reshape(-1, 1)
    c["bn2g"] = inputs["bn2_g"].astype(f32).reshape(-1, 1)
    c["bn2b"] = inputs["bn2_b"].astype(f32).reshape(-1, 1)
    return c


def _build(NCHT, VALT, NCHE, repeat_k=1, single_core=False, no_coll=False,
           no_gather=False):
    from concourse import bacc, bass, mybir, tile

    f32 = mybir.dt.float32
    bf16 = mybir.dt.bfloat16
    i16 = mybir.dt.int16
    nc = bacc.Bacc(
        "TRN2",
        target_bir_lowering=False,
        debug=False,
        enable_asserts=False,
        num_devices=1 if single_core else M,
        num_swdge_queues=4,
    )
    local_coll = single_core or no_coll  # replace collectives w/ local copies

    EPTT = [v * P for v in NCHT]
    TOT = sum(EPTT)
    TOTC = sum(NCHT)
    EOFF = np.concatenate([[0], np.cumsum(EPTT)]).astype(int)
    COFF = np.concatenate([[0], np.cumsum(NCHT)]).astype(int)

    def din(name, shape, dtype=f32):
        return nc.dram_tensor(name, list(shape), dtype, kind="ExternalInput")

    fp8 = mybir.dt.float8e4
    xin = din("xin", (NPC, 64))  # per-core slice of x
    idx_src = din("idx_src", (128, TOT // 16), i16)
    dstloc = din("dstloc", (P, TOTC))
    otall = din("otall", (P, TOT), fp8)
    iotab = din("iotab", (P, P))
    ident = din("ident", (P, P))
    ones = din("ones", (P, 1))
    ww = [din(f"ww{l + 1}", (LAYERS[l]["Fin"], HEADS * LAYERS[l]["C"] + 16)) for l in range(4)]
    bb = [
        din("bb1", (P, 128)),
        din("bb2", (P, 32)),
        din("bb3", (P, 128)),
        din("bb4", (P, 64)),
    ]
    bng = [None, din("bn1g", (128, 1)), None, din("bn2g", (128, 1))]
    bnb = [None, din("bn1b", (128, 1)), None, din("bn2b", (128, 1))]
    out_ext = nc.dram_tensor("out", [NPC, 64], f32, kind="ExternalOutput")

    with tile.TileContext(nc) as tc:
        with (
            tc.tile_pool(name="dram", bufs=1, space="DRAM") as dram,
            tc.tile_pool(name="const", bufs=1) as cpool,
            tc.tile_pool(name="work", bufs=3) as wpool,
            tc.tile_pool(name="gath", bufs=2) as gpool,
            tc.tile_pool(name="psum", bufs=2, space="PSUM") as ppool,
        ):
            # ---- internal DRAM (local) ----
            tabL = [
                dram.tile([NPC, LAYERS[l]["R2"]], bf16, tag=f"tabL{l}", name=f"tabL{l}")
                for l in range(4)
            ]
            sdo = dram.tile([NPC, 8], f32, tag="sdo", name="sdo")
            own = [
                dram.tile([NPC, OWNW[l]], f32, tag=f"own{l}", name=f"own{l}")
                for l in range(3)
            ]
            ostat = [
                dram.tile([128, 2], f32, tag="ostat0", name="ostat0"),
                None,
                dram.tile([128, 2], f32, tag="ostat2", name="ostat2"),
            ]

            def fresh_shared(rep):
                sfx = "" if rep == 0 else f"r{rep}"
                aspace = {} if local_coll else dict(addr_space="Shared")
                tabA = [
                    dram.tile([N, LAYERS[l]["R2"]], bf16, tag=f"tabA{l}{sfx}",
                              name=f"tabA{l}{sfx}", **aspace)
                    for l in range(4)
                ]
                gstat = [
                    dram.tile([128, 2], f32, tag=f"gs0{sfx}", name=f"gs0{sfx}", **aspace),
                    None,
                    dram.tile([128, 2], f32, tag=f"gs2{sfx}", name=f"gs2{sfx}", **aspace),
                ]
                return tabA, gstat

            # ---- consts to SBUF ----
            def load_const(ap, shape, dtype=f32, tag=None):
                t = cpool.tile(list(shape), dtype, tag=tag, name=tag)
                nc.sync.dma_start(out=t[:], in_=ap[:])
                return t

            iotaf_sb = load_const(iotab, (P, P), tag="iotaf")
            ident_sb = load_const(ident, (P, P), tag="ident")
            ones_sb = load_const(ones, (P, 1), tag="ones")
            isrc_sb = load_const(idx_src, (128, TOT // 16), i16, tag="isrc")
            dstloc_sb = load_const(dstloc, (P, TOTC), tag="dstloc")
            ww_sb = [
                load_const(ww[l], (LAYERS[l]["Fin"], HEADS * LAYERS[l]["C"] + 16), tag=f"ww{l}")
                for l in range(4)
            ]
            bb_sb = [
                load_const(bb[0], (P, 128), tag="bb0"),
                load_const(bb[1], (P, 32), tag="bb1"),
                load_const(bb[2], (P, 128), tag="bb2"),
                load_const(bb[3], (P, 64), tag="bb3"),
            ]
            bng_sb = [None, load_const(bng[1], (128, 1), tag="bng1"), None, load_const(bng[3], (128, 1), tag="bng3")]
            bnb_sb = [None, load_const(bnb[1], (128, 1), tag="bnb1"), None, load_const(bnb[3], (128, 1), tag="bnb3")]

            AX = mybir.AxisListType.X
            OP = mybir.AluOpType
            AF = mybir.ActivationFunctionType

            iotab_sb = cpool.tile([P, P], bf16, tag="iotabf", name="iotabf")
            nc.vector.tensor_copy(iotab_sb[:], iotaf_sb[:])

            # stream the fp8 transposed one-hot into a persistent bf16 buffer
            # (small staging tile; conversion on the idle Act engine)
            otall_sb = cpool.tile([P, TOT], bf16, tag="otall", name="otall")
            OTCH = TOT // 8
            for ci in range(8):
                stg = wpool.tile([P, OTCH], fp8, tag="otstage", name="otstage", bufs=2)
                nc.sync.dma_start(out=stg[:], in_=otall[:, ci * OTCH : (ci + 1) * OTCH])
                nc.scalar.activation(
                    otall_sb[:, ci * OTCH : (ci + 1) * OTCH], stg[:], AF.Identity
                )

            # registers holding valid-index counts for dma_gather
            ept_regs = {}
            for v in sorted(set(VALT)):
                r = nc.alloc_registers(name=f"ept{v}")
                nc.regs_mov(r, v)
                ept_regs[v] = nc.snap(r, donate=False)



            # ============ node phase (own 2500 rows only) ============
            def node_phase(l, src_dram, gstat):
                cfg = LAYERS[l]
                Fin, C, R2 = cfg["Fin"], cfg["C"], cfg["R2"]
                HC = HEADS * C
                scale_off = None
                if cfg["bn"]:
                    sg = wpool.tile([128, 2], f32, tag="sg", name="sg")
                    nc.sync.dma_start(out=sg[:], in_=gstat[l - 1][:])
                    mu = wpool.tile([Fin, 1], f32, tag="mu", name="mu")
                    nc.vector.tensor_scalar(mu[:], sg[:, 0:1], 1.0 / N, None, OP.mult)
                    msq = wpool.tile([Fin, 1], f32, tag="msq", name="msq")
                    nc.vector.tensor_scalar(msq[:], sg[:, 1:2], 1.0 / N, None, OP.mult)
                    var = wpool.tile([Fin, 1], f32, tag="var", name="var")
                    nc.vector.tensor_tensor(var[:], mu[:], mu[:], OP.mult)
                    nc.vector.tensor_tensor(var[:], msq[:], var[:], OP.subtract)
                    nc.vector.tensor_scalar(var[:], var[:], BN_EPS, None, OP.add)
                    sdv = wpool.tile([Fin, 1], f32, tag="sdv", name="sdv")
                    nc.scalar.activation(sdv[:], var[:], AF.Sqrt)
                    rs = wpool.tile([Fin, 1], f32, tag="rs", name="rs")
                    nc.vector.reciprocal(rs[:], sdv[:])
                    bscale = wpool.tile([Fin, 1], f32, tag="bscale", name="bscale")
                    nc.vector.tensor_tensor(bscale[:], rs[:], bng_sb[l][:], OP.mult)
                    boff = wpool.tile([Fin, 1], f32, tag="boff", name="boff")
                    nc.vector.tensor_tensor(boff[:], mu[:], bscale[:], OP.mult)
                    nc.vector.tensor_tensor(boff[:], bnb_sb[l][:], boff[:], OP.subtract)
                    scale_off = (bscale, boff)

                for t in range(NT):
                    cnt = min(P, NPC - t * P)
                    xt = wpool.tile([P, Fin], f32, tag="xt", name="xt")
                    if cnt < P:
                        nc.vector.memset(xt[:], 0.0)
                    nc.sync.dma_start(out=xt[:cnt, :], in_=src_dram[t * P : t * P + cnt, :Fin])
                    xtp = ppool.tile([Fin, P], f32, tag="xtp", name="xtp", bufs=2)
                    nc.tensor.transpose(out=xtp[:], in_=xt[:], identity=ident_sb[:])
                    xts = wpool.tile([Fin, P], f32, tag="xts", name="xts")
                    if scale_off is not None:
                        # fused BN affine + relu on the Act engine
                        nc.scalar.activation(
                            xts[:], xtp[:], AF.Relu,
                            bias=scale_off[1][:], scale=scale_off[0][:],
                        )
                    else:
                        nc.scalar.activation(xts[:], xtp[:], AF.Identity)
                    tt = wpool.tile([P, HC + 8], bf16, tag="tt", name="tt")
                    sd = wpool.tile([P, 8], f32, tag="sd", name="sd")
                    if HC + 16 <= 512:
                        hp = ppool.tile([P, HC + 16], f32, tag="pmm", name="hp", bufs=2)
                        nc.tensor.matmul(out=hp[:], lhsT=xts[:], rhs=ww_sb[l][:], start=True, stop=True)
                        nc.vector.tensor_copy(tt[:], hp[:, : HC + 8])
                        nc.vector.tensor_copy(sd[:], hp[:, HC + 8 : HC + 16])
                    else:  # L4: 528 cols -> split 512 + 16
                        hp = ppool.tile([P, 512], f32, tag="pmm", name="hp", bufs=2)
                        hp2 = ppool.tile([P, 16], f32, tag="pmm2", name="hp2", bufs=1)
                        nc.tensor.matmul(out=hp[:], lhsT=xts[:], rhs=ww_sb[l][:, :512], start=True, stop=True)
                        nc.tensor.matmul(out=hp2[:], lhsT=xts[:], rhs=ww_sb[l][:, 512:], start=True, stop=True)
                        nc.vector.tensor_copy(tt[:, :512], hp[:])
                        nc.vector.tensor_copy(tt[:, 512:520], hp2[:, 0:8])
                        nc.vector.tensor_copy(sd[:], hp2[:, 8:16])
                    nc.sync.dma_start(
                        out=tabL[l][t * P : t * P + cnt, : HC + 8], in_=tt[:cnt, :]
                    )
                    nc.sync.dma_start(out=sdo[t * P : t * P + cnt, :], in_=sd[:cnt, :])

            # ============ edge phase (own dst tiles) ============
            def edge_phase(l, tabA, out_dram):
                cfg = LAYERS[l]
                C, R2 = cfg["C"], cfg["R2"]
                HC = HEADS * C
                do_stat = l in (0, 2)
                if do_stat:
                    accS = wpool.tile([HC, 2], f32, tag="accS", name="accS")
                    nc.vector.memset(accS[:], 0.0)
                for t in range(NT):
                    NCH = NCHE[t]  # effective chunks (real edges only)
                    cnt = min(P, NPC - t * P)
                    # one-hot O for all chunks of this tile (single DVE op);
                    # pad edges have dstloc=999 -> all-zero column
                    Oall = gpool.tile([P, NCH * P], bf16, tag="Oall", name="Oall")
                    nc.vector.tensor_tensor(
                        Oall[:].rearrange("p (j f) -> p j f", f=P),
                        iotab_sb[:].unsqueeze(1).to_broadcast((P, NCH, P)),
                        dstloc_sb[:, COFF[t] : COFF[t] + NCH].unsqueeze(2).to_broadcast((P, NCH, P)),
                        OP.is_equal,
                    )
                    # own s_dst rows for this tile (local, static offset)
                    sdtf = wpool.tile([P, 8], f32, tag="sdtf", name="sdtf")
                    if cnt < P:
                        nc.vector.memset(sdtf[:], 0.0)
                    nc.sync.dma_start(out=sdtf[:cnt, :], in_=sdo[t * P : t * P + cnt, :])
                    sdtb = wpool.tile([P, 8], bf16, tag="sdtb", name="sdtb")
                    nc.vector.tensor_copy(sdtb[:], sdtf[:])
                    # per-chunk SD[e,h] = s_dst[dst(e),h] via matmul with the
                    # host-precomputed transposed one-hot (no PE transposes)
                    psSD = ppool.tile([P, NCH * 8], f32, tag="psSD", name="psSD", bufs=1)
                    for j in range(NCH):
                        blk = (COFF[t] + j) * P
                        nc.tensor.matmul(
                            out=psSD[:, j * 8 : (j + 1) * 8],
                            lhsT=otall_sb[:, blk : blk + P], rhs=sdtb[:],
                            start=True, stop=True,
                        )
                    # gather source rows [h | s_src] (bf16) from the global table
                    G = gpool.tile([P, NCH * R2], bf16, tag="G", name="G")
                    if not no_gather:
                        nc.gpsimd.dma_gather(
                            out_ap=G[:].rearrange("p (j r) -> p j r", r=R2),
                            in_ap=tabA[l][:],
                            idxs_ap=isrc_sb[
                                :, EOFF[t] // 16 : EOFF[t] // 16 + NCH * 8
                            ],
                            num_idxs=NCH * P,
                            num_idxs_reg=ept_regs[VALT[t]],
                            elem_size=R2,
                            single_packet=False,
                            queue_num=t % 4,
                        )
                    else:
                        nc.vector.memset(G[:, :8], 0.0)
                    G3 = G[:].rearrange("p (j r) -> p j r", r=R2)
                    # batched e = LeakyReLU(s_src + s_dst); EXS = exp(e) (bf16)
                    EB = wpool.tile([P, NCH * 8], f32, tag="EB", name="EB")
                    nc.vector.tensor_tensor(
                        EB[:].rearrange("p (j r) -> p j r", r=8),
                        G3[:, :, HC : HC + 8],
                        psSD[:].rearrange("p (j r) -> p j r", r=8),
                        OP.add,
                    )
                    EB2 = wpool.tile([P, NCH * 8], f32, tag="EB2", name="EB2")
                    nc.vector.tensor_scalar(EB2[:], EB[:], NEG, None, OP.mult)
                    nc.vector.tensor_tensor(EB[:], EB[:], EB2[:], OP.max)
                    EXS = wpool.tile([P, NCH * 8], bf16, tag="EXS", name="EXS")
                    nc.scalar.activation(EXS[:], EB[:], AF.Exp)
                    # h *= exp(e) in place on G (one 4D DVE op for all chunks)
                    nc.vector.tensor_tensor(
                        G3[:, :, :HC].rearrange("p j (h c) -> p j h c", c=C),
                        G3[:, :, :HC].rearrange("p j (h c) -> p j h c", c=C),
                        EXS[:].rearrange("p (j h) -> p j h", h=8)
                        .unsqueeze(3).to_broadcast((P, NCH, 8, C)),
                        OP.mult,
                    )
                    # overwrite the s_src columns with exp(e) (denominator)
                    nc.scalar.activation(
                        G3[:, :, HC : HC + 8],
                        EXS[:].rearrange("p (j r) -> p j r", r=8),
                        AF.Identity,
                    )
                    # per-chunk weighted scatter straight out of G
                    if HC + 8 <= 512:
                        psA = ppool.tile([P, HC + 8], f32, tag="pmm", name="psA", bufs=2)
                        psB = None
                    else:
                        psA = ppool.tile([P, 512], f32, tag="pmm", name="psA", bufs=2)
                        psB = ppool.tile([P, 8], f32, tag="pmm2", name="psB", bufs=1)
                    for j in range(NCH):
                        if psB is None:
                            nc.tensor.matmul(
                                out=psA[:], lhsT=Oall[:, j * P : (j + 1) * P],
                                rhs=G3[:, j, : HC + 8],
                                start=(j == 0), stop=(j == NCH - 1),
                            )
                        else:
                            nc.tensor.matmul(
                                out=psA[:], lhsT=Oall[:, j * P : (j + 1) * P],
                                rhs=G3[:, j, :512],
                                start=(j == 0), stop=(j == NCH - 1),
                            )
                            nc.tensor.matmul(
                                out=psB[:], lhsT=Oall[:, j * P : (j + 1) * P],
                                rhs=G3[:, j, 512 : HC + 8],
                                start=(j == 0), stop=(j == NCH - 1),
                            )
                    den = psA[:, HC : HC + 8] if psB is None else psB[:]
                    rec = wpool.tile([P, 8], f32, tag="rec", name="rec")
                    nc.vector.tensor_scalar(rec[:], den, 1e-16, None, OP.add)
                    nc.vector.reciprocal(rec[:], rec[:])
                    res = wpool.tile([P, HC], f32, tag="res", name="res")
                    nc.vector.tensor_tensor(
                        res[:].rearrange("p (h c) -> p h c", h=HEADS),
                        psA[:, :HC].rearrange("p (h c) -> p h c", h=HEADS),
                        rec[:].unsqueeze(2).to_broadcast((P, HEADS, C)),
                        OP.mult,
                    )
                    if cfg["concat"]:
                        nc.vector.tensor_tensor(res[:], res[:], bb_sb[l][:], OP.add)
                        nc.sync.dma_start(
                            out=out_dram[t * P : t * P + cnt, :], in_=res[:cnt, :]
                        )
                        if do_stat:
                            sq = wpool.tile([P, HC], f32, tag="sq", name="sq")
                            nc.scalar.square(sq[:], res[:])
                            psS1 = ppool.tile([HC, 1], f32, tag="psS", name="psS1", bufs=2)
                            nc.tensor.matmul(
                                out=psS1[:], lhsT=res[:cnt, :], rhs=ones_sb[:cnt, :],
                                start=True, stop=True,
                            )
                            nc.vector.tensor_tensor(accS[:, 0:1], accS[:, 0:1], psS1[:], OP.add)
                            psS2 = ppool.tile([HC, 1], f32, tag="psS", name="psS2", bufs=2)
                            nc.tensor.matmul(
                                out=psS2[:], lhsT=sq[:cnt, :], rhs=ones_sb[:cnt, :],
                                start=True, stop=True,
                            )
                            nc.vector.tensor_tensor(accS[:, 1:2], accS[:, 1:2], psS2[:], OP.add)
                    else:
                        red = wpool.tile([P, C], f32, tag="red", name="red")
                        nc.vector.tensor_reduce(
                            red[:],
                            res[:].rearrange("p (h c) -> p c h", h=HEADS),
                            AX,
                            OP.add,
                        )
                        nc.vector.tensor_scalar(red[:], red[:], 1.0 / HEADS, None, OP.mult)
                        nc.vector.tensor_tensor(red[:], red[:], bb_sb[l][:, :C], OP.add)
                        nc.sync.dma_start(
                            out=out_dram[t * P : t * P + cnt, :], in_=red[:cnt, :]
                        )
                if do_stat:
                    nc.sync.dma_start(out=ostat[l][:], in_=accS[:])

            # ================= full pipeline =================
            for _rep in range(repeat_k):
                tabA, gstat = fresh_shared(_rep)
                # chain reps through out_ext so repeat_k>1 timing can't be
                # dead-code-eliminated (used for timing only; output = model^k)
                srcs = [xin if _rep == 0 else out_ext, own[0], own[1], own[2]]
                outs = [own[0], own[1], own[2], out_ext]
                for l in range(4):
                    node_phase(l, srcs[l], gstat)
                    if local_coll:
                        nc.sync.dma_start(out=tabA[l][0:NPC, :], in_=tabL[l][:, :])
                    else:
                        nc.gpsimd.collective_compute(
                            "AllGather",
                            mybir.AluOpType.bypass,
                            replica_groups=[list(range(M))],
                            ins=[tabL[l].opt()],
                            outs=[tabA[l].opt()],
                        )
                    edge_phase(l, tabA, outs[l])
                    if l in (0, 2):
                        if local_coll:
                            nc.sync.dma_start(out=gstat[l][:, :], in_=ostat[l][:, :])
                        else:
                            nc.gpsimd.collective_compute(
                                "AllReduce",
                                mybir.AluOpType.add,
                                replica_groups=[list(range(M))],
                                ins=[ostat[l].opt()],
                                outs=[gstat[l].opt()],
                            )
    if not nc.is_finalized():
        nc.finalize()
    return nc


def _pjrt_exec(nc, in_maps, time_reps=0):
    """Mirror of bass2jax.run_bass_via_pjrt multi-core path, holding the jitted
    executable so repeated executions can be wall-timed."""
    import time as _t
    import jax
    from jax.experimental.shard_map import shard_map
    from jax.sharding import Mesh, PartitionSpec
    from concourse import bass2jax as B, mybir as mb

    B.install_neuronx_cc_hook()
    n_cores = len(in_maps)
    partition_name = nc.partition_id_tensor.name if nc.partition_id_tensor else None
    in_names, out_names, out_avals, zero_outs = [], [], [], []
    for alloc in nc.m.functions[0].allocations:
        if not isinstance(alloc, mb.MemoryLocationSet):
            continue
        name = alloc.memorylocations[0].name
        if alloc.kind == "ExternalInput":
            if name != partition_name:
                in_names.append(name)
        elif alloc.kind == "ExternalOutput":
            out_names.append(name)
            shape = tuple(alloc.tensor_shape)
            dtype = mb.dt.np(alloc.dtype)
            out_avals.append(jax.core.ShapedArray(shape, dtype))
            zero_outs.append(np.zeros(shape, dtype))
    n_params = len(in_names)
    n_outs = len(out_avals)
    in_names.extend(out_names)
    if partition_name is not None:
        in_names.append(partition_name)
    donate = tuple(range(n_params, n_params + n_outs))

    def _body(*args):
        operands = list(args)
        if partition_name is not None:
            operands.append(B.partition_id_tensor())
        outs = B._bass_exec_p.bind(
            *operands,
            out_avals=tuple(out_avals),
            in_names=tuple(in_names),
            out_names=tuple(out_names),
            lowering_input_output_aliases=(),
            sim_require_finite=True,
            sim_require_nnan=True,
            nc=nc,
        )
        return tuple(outs)

    devices = jax.devices()[:n_cores]
    mesh = Mesh(np.asarray(devices), ("core",))
    in_specs = (PartitionSpec("core"),) * (n_params + n_outs)
    out_specs = (PartitionSpec("core"),) * len(out_names)
    sharded = jax.jit(
        shard_map(_body, mesh=mesh, in_specs=in_specs, out_specs=out_specs,
                  check_rep=False),
        donate_argnums=donate, keep_unused=True,
    )
    per_core = [[np.asarray(m_[nm]) for nm in in_names[:n_params]] for m_ in in_maps]
    concat_in = [
        np.concatenate([per_core[c][i] for c in range(n_cores)], axis=0)
        for i in range(n_params)
    ]
    from jax.sharding import NamedSharding
    shard = NamedSharding(mesh, PartitionSpec("core"))
    concat_in = [jax.device_put(a, shard) for a in concat_in]
    jax.block_until_ready(concat_in)

    def once():
        cz = [jax.device_put(np.zeros((n_cores * z.shape[0], *z.shape[1:]), z.dtype), shard)
              for z in zero_outs]
        jax.block_until_ready(cz)
        t0 = _t.perf_counter()
        out_arrs = sharded(*concat_in, *cz)
        jax.block_until_ready(out_arrs)
        return _t.perf_counter() - t0, out_arrs

    _, out_arrs = once()  # compile + first run
    times = []
    for _ in range(time_reps):
        dt, out_arrs = once()
        times.append(dt)
    res = [
        {nm: np.asarray(out_arrs[i]).reshape(n_cores, *out_avals[i].shape)[c]
         for i, nm in enumerate(out_names)}
        for c in range(n_cores)
    ]
    # free device buffers so back-to-back _run calls don't accumulate HBM use
    import gc

    for a in list(concat_in) + list(out_arrs):
        try:
            a.delete()
        except Exception:
            pass
    del concat_in, out_arrs
    gc.collect()
    return res, (min(times) if times else None)


def _run(inputs, trace=False, time_reps=0, repeat_k=1, no_coll=False, no_gather=False):
    NCHT, VALT, NCHE, edata = _preprocess(np.asarray(inputs["edge_index"]))
    consts = _host_consts(inputs)
    nc = _build(NCHT, VALT, NCHE, repeat_k=repeat_k, no_coll=no_coll,
                no_gather=no_gather)

    x = np.asarray(inputs["x"], dtype=np.float32)
    in_maps = []
    for m in range(M):
        d = dict(consts)
        d.update(edata[m])
        d["xin"] = np.ascontiguousarray(x[m * NPC : (m + 1) * NPC])
        in_maps.append(d)

    if time_reps > 0:
        results, best_s = _pjrt_exec(nc, in_maps, time_reps=time_reps)
    else:
        from concourse.bass_utils import run_bass_kernel_spmd

        res = run_bass_kernel_spmd(nc, in_maps, core_ids=list(range(M)))
        results, best_s = res.results, None
    outs = [np.asarray(results[m]["out"]) for m in range(M)]
    full = np.concatenate(outs, axis=0).astype(np.float32)
    return full, (None if best_s is None else int(best_s * 1e9))


def kernel(**inputs):
    out, _ = _run(inputs, trace=False)
    return out



# revision 39
# speedup vs baseline: 46.2455x; 1.3671x over previous
"""Distributed Bass kernel for a 4-layer GAT autoencoder on 8 TRN2 NeuronCores.

Strategy (per sharding hint): nodes sharded across 8 cores (2500/core);
edges co-located with their destination node's core, sorted by destination;
params replicated.

v3 layout: the node phase is SHARDED — each core transforms only its own
2500 rows (inputs pre-sliced per core host-side, so all node-phase DMA uses
static local offsets), then the compact bf16 per-node tables [h | s_src]
are AllGathered so the edge phase can gather any source row. s_dst stays
local (a [2500,8] buffer). Pad edges carry an out-of-range dstloc (999), so
their one-hot column is all zeros and they contribute nothing to the
scatter or the softmax denominator — no NEGBIG dummy-row machinery.

Edge phase per dst tile: one-hot O for all chunks in one DVE op; a big bf16
dma_gather of source rows; per-edge s_dst via PE transpose(O_j) + an
8-column matmul; batched LeakyReLU/exp; per-chunk h*ex one-hot scatter
matmuls accumulating numerator and denominator in PSUM. BN statistics are
per-core partial sums (closed-group matmuls + SBUF accumulate) + a [128,2]
AllReduce; there is no separate stats pass.
"""

import sys

sys.path.insert(0, "/opt/trn_rl_repo")

import numpy as np

P = 128
M = 8
N = 20000
NPC = N // M  # 2500 nodes per core
NT = (NPC + P - 1) // P  # 20 dst tiles per core
HEADS = 8
NEG = 0.2
BN_EPS = 1e-5
PADDST = 999.0  # out-of-range dst slot for pad edges -> zero one-hot column

# layer configs: Fin, C (per-head out), concat?, bn on input?, bf16 row width R2
LAYERS = [
    dict(Fin=64, C=16, concat=True, bn=False, R2=256),
    dict(Fin=128, C=32, concat=False, bn=True, R2=384),
    dict(Fin=32, C=16, concat=True, bn=False, R2=256),
    dict(Fin=128, C=64, concat=False, bn=True, R2=640),
]
OWNW = [128, 32, 128, 64]  # own[l] row widths


def _wrap16(idx):
    """Host int array -> dma_gather index layout [16, n/16] (idx[s*16+p] at [p,s])."""
    n = idx.shape[0]
    assert n % 16 == 0
    w = np.ascontiguousarray(idx.reshape(n // 16, 16).T).astype(np.int16)
    return np.ascontiguousarray(np.tile(w, (8, 1)))  # replicated for the 8 Q7 cores


def _preprocess(edge_index):
    """Partition + sort edges; per-tile chunk counts; per-core gather indices."""
    src = np.concatenate([np.asarray(edge_index[0]), np.arange(N)]).astype(np.int64)
    dst = np.concatenate([np.asarray(edge_index[1]), np.arange(N)]).astype(np.int64)

    per_core = []
    cnts = np.zeros((M, NT), dtype=np.int64)
    for m in range(M):
        sel = (dst // NPC) == m
        s, d = src[sel], dst[sel]
        dloc = d - NPC * m
        order = np.argsort(dloc, kind="stable")
        s, dloc = s[order], dloc[order]
        tiles = []
        for t in range(NT):
            tsel = (dloc // P) == t
            st, dt_ = s[tsel], dloc[tsel] - t * P
            tiles.append((st, dt_))
            cnts[m, t] = st.shape[0]
        per_core.append(tiles)

    # per-tile chunk count: max over cores, rounded up to a multiple of 4
    # (keeps every tile's idx segment 64B-aligned for the dma_gather ucode)
    nch = np.maximum(((cnts.max(axis=0) + P - 1) // P + 3) // 4 * 4, 4)
    NCHT = [int(v) for v in nch]
    EPTT = [v * P for v in NCHT]
    TOT = int(sum(EPTT))

    # effective chunk count per tile: only chunks holding real edges (the
    # x4-rounded layout keeps its offsets; trailing all-pad chunks are simply
    # never gathered or processed). All gathered slots use idx>=0 (pads gather
    # row 0) -- negative-index skipping produced garbage on HW.
    NCHE = [min((int(cnts.max(axis=0)[t]) + P - 1) // P, NCHT[t]) for t in range(NT)]
    VALT = [NCHE[t] * P for t in range(NT)]

    data = []
    for m in range(M):
        isrc = np.zeros((TOT,), dtype=np.int64)  # pad edges gather row 0
        dloc_cols = np.full((P, sum(NCHT)), PADDST, dtype=np.float32)
        eoff = 0
        coff = 0
        for t in range(NT):
            st, dt_ = per_core[m][t]
            c = st.shape[0]
            isrc[eoff : eoff + c] = st
            dl = np.full((EPTT[t],), PADDST, dtype=np.float64)
            dl[:c] = dt_
            # column coff+j, row p  = edge (t, j*128+p)
            dloc_cols[:, coff : coff + NCHT[t]] = dl.reshape(NCHT[t], P).T
            eoff += EPTT[t]
            coff += NCHT[t]
        # host-precomputed transposed one-hot: OT[n, c*128+q] = 1 iff edge
        # (chunk col c, slot q) targets local node n  (pad cols all-zero).
        # shipped fp8 (0/1 exact) to halve the input transfer; converted to
        # bf16 once on-chip.
        import ml_dtypes

        TOTC = sum(NCHT)
        OT = np.zeros((P, TOTC * P), dtype=ml_dtypes.float8_e4m3)
        d_ = dloc_cols.astype(np.int64)  # [q, c]
        q_idx, c_idx = np.nonzero(d_ < P)
        OT[d_[q_idx, c_idx], c_idx * P + q_idx] = 1
        data.append(
            dict(idx_src=_wrap16(isrc), dstloc=np.ascontiguousarray(dloc_cols),
                 otall=OT)
        )
    return NCHT, VALT, NCHE, data


def _host_consts(inputs):
    """Fused weights + broadcast biases + misc consts."""
    f32 = np.float32
    c = {}
    c["iotab"] = np.tile(np.arange(P, dtype=f32)[None, :], (P, 1))
    c["ident"] = np.eye(P, dtype=f32)
    c["ones"] = np.ones((P, 1), dtype=f32)

    def fuse(W, a_s, a_d):
        # WW = [W | W@blockdiag(a_src) | W@blockdiag(a_dst)]  -> [Fin, HC+16]
        H, C_ = a_s.shape
        Ws = np.einsum("fhc,hc->fh", W.reshape(-1, H, C_), a_s)
        Wd = np.einsum("fhc,hc->fh", W.reshape(-1, H, C_), a_d)
        return np.concatenate([W, Ws, Wd], axis=1).astype(f32)

    c["ww1"] = fuse(inputs["We1"], inputs["as_e1"], inputs["ad_e1"])
    c["ww2"] = fuse(inputs["We2"], inputs["as_e2"], inputs["ad_e2"])
    c["ww3"] = fuse(inputs["Wd1"], inputs["as_d1"], inputs["ad_d1"])
    c["ww4"] = fuse(inputs["Wd2"], inputs["as_d2"], inputs["ad_d2"])
    c["bb1"] = np.tile(inputs["b_e1"][None, :], (P, 1)).astype(f32)  # [128,128]
    c["bb2"] = np.tile(inputs["b_e2"][None, :], (P, 1)).astype(f32)  # [128,32]
    c["bb3"] = np.tile(inputs["b_d1"][None, :], (P, 1)).astype(f32)  # [128,128]
    c["bb4"] = np.tile(inputs["b_d2"][None, :], (P, 1)).astype(f32)  # [128,64]
    c["bn1g"] = inputs["bn1_g"].astype(f32).reshape(-1, 1)  # [128,1]
    c["bn1b"] = inputs["bn1_b"].astype(f32).reshape(-1, 1)
    c["bn2g"] = inputs["bn2_g"].astype(f32).reshape(-1, 1)
    c["bn2b"] = inputs["bn2_b"].astype(f32).reshape(-1, 1)
    return c


def _build(NCHT, VALT, NCHE, repeat_k=1, single_core=False, no_coll=False,
           no_gather=False):
    from concourse import bacc, bass, mybir, tile

    f32 = mybir.dt.float32
    bf16 = mybir.dt.bfloat16
    i16 = mybir.dt.int16
    nc = bacc.Bacc(
        "TRN2",
        target_bir_lowering=False,
        debug=False,
        enable_asserts=False,
        num_devices=1 if single_core else M,
        num_swdge_queues=4,
    )
    local_coll = single_core or no_coll  # replace collectives w/ local copies

    EPTT = [v * P for v in NCHT]
    TOT = sum(EPTT)
    TOTC = sum(NCHT)
    EOFF = np.concatenate([[0], np.cumsum(EPTT)]).astype(int)
    COFF = np.concatenate([[0], np.cumsum(NCHT)]).astype(int)

    def din(name, shape, dtype=f32):
        return nc.dram_tensor(name, list(shape), dtype, kind="ExternalInput")

    fp8 = mybir.dt.float8e4
    xin = din("xin", (NPC, 64))  # per-core slice of x
    idx_src = din("idx_src", (128, TOT // 16), i16)
    dstloc = din("dstloc", (P, TOTC))
    otall = din("otall", (P, TOT), fp8)
    iotab = din("iotab", (P, P))
    ident = din("ident", (P, P))
    ones = din("ones", (P, 1))
    ww = [din(f"ww{l + 1}", (LAYERS[l]["Fin"], HEADS * LAYERS[l]["C"] + 16)) for l in range(4)]
    bb = [
        din("bb1", (P, 128)),
        din("bb2", (P, 32)),
        din("bb3", (P, 128)),
        din("bb4", (P, 64)),
    ]
    bng = [None, din("bn1g", (128, 1)), None, din("bn2g", (128, 1))]
    bnb = [None, din("bn1b", (128, 1)), None, din("bn2b", (128, 1))]
    out_ext = nc.dram_tensor("out", [NPC, 64], f32, kind="ExternalOutput")

    with tile.TileContext(nc) as tc:
        with (
            tc.tile_pool(name="dram", bufs=1, space="DRAM") as dram,
            tc.tile_pool(name="const", bufs=1) as cpool,
            tc.tile_pool(name="work", bufs=3) as wpool,
            tc.tile_pool(name="gath", bufs=2) as gpool,
            tc.tile_pool(name="psum", bufs=2, space="PSUM") as ppool,
        ):
            # ---- internal DRAM (local) ----
            tabL = [
                dram.tile([NPC, LAYERS[l]["R2"]], bf16, tag=f"tabL{l}", name=f"tabL{l}")
                for l in range(4)
            ]
            sdo = dram.tile([NPC, 8], f32, tag="sdo", name="sdo")
            own = [
                dram.tile([NPC, OWNW[l]], f32, tag=f"own{l}", name=f"own{l}")
                for l in range(3)
            ]
            ostat = [
                dram.tile([128, 2], f32, tag="ostat0", name="ostat0"),
                None,
                dram.tile([128, 2], f32, tag="ostat2", name="ostat2"),
            ]

            def fresh_shared(rep):
                sfx = "" if rep == 0 else f"r{rep}"
                aspace = {} if local_coll else dict(addr_space="Shared")
                tabA = [
                    dram.tile([N, LAYERS[l]["R2"]], bf16, tag=f"tabA{l}{sfx}",
                              name=f"tabA{l}{sfx}", **aspace)
                    for l in range(4)
                ]
                gstat = [
                    dram.tile([128, 2], f32, tag=f"gs0{sfx}", name=f"gs0{sfx}", **aspace),
                    None,
                    dram.tile([128, 2], f32, tag=f"gs2{sfx}", name=f"gs2{sfx}", **aspace),
                ]
                return tabA, gstat

            # ---- consts to SBUF ----
            def load_const(ap, shape, dtype=f32, tag=None):
                t = cpool.tile(list(shape), dtype, tag=tag, name=tag)
                nc.sync.dma_start(out=t[:], in_=ap[:])
                return t

            iotaf_sb = load_const(iotab, (P, P), tag="iotaf")
            ident_sb = load_const(ident, (P, P), tag="ident")
            ones_sb = load_const(ones, (P, 1), tag="ones")
            isrc_sb = load_const(idx_src, (128, TOT // 16), i16, tag="isrc")
            dstloc_sb = load_const(dstloc, (P, TOTC), tag="dstloc")
            ww_sb = [
                load_const(ww[l], (LAYERS[l]["Fin"], HEADS * LAYERS[l]["C"] + 16), tag=f"ww{l}")
                for l in range(4)
            ]
            bb_sb = [
                load_const(bb[0], (P, 128), tag="bb0"),
                load_const(bb[1], (P, 32), tag="bb1"),
                load_const(bb[2], (P, 128), tag="bb2"),
                load_const(bb[3], (P, 64), tag="bb3"),
            ]
            bng_sb = [None, load_const(bng[1], (128, 1), tag="bng1"), None, load_const(bng[3], (128, 1), tag="bng3")]
            bnb_sb = [None, load_const(bnb[1], (128, 1), tag="bnb1"), None, load_const(bnb[3], (128, 1), tag="bnb3")]

            AX = mybir.AxisListType.X
            OP = mybir.AluOpType
            AF = mybir.ActivationFunctionType

            iotab_sb = cpool.tile([P, P], bf16, tag="iotabf", name="iotabf")
            nc.vector.tensor_copy(iotab_sb[:], iotaf_sb[:])

            # stream the fp8 transposed one-hot into a persistent bf16 buffer
            # (small staging tile; conversion on the idle Act engine)
            otall_sb = cpool.tile([P, TOT], bf16, tag="otall", name="otall")
            OTCH = TOT // 8
            for ci in range(8):
                stg = wpool.tile([P, OTCH], fp8, tag="otstage", name="otstage", bufs=2)
                nc.sync.dma_start(out=stg[:], in_=otall[:, ci * OTCH : (ci + 1) * OTCH])
                nc.scalar.activation(
                    otall_sb[:, ci * OTCH : (ci + 1) * OTCH], stg[:], AF.Identity
                )

            # registers holding valid-index counts for dma_gather
            QG = 4  # chunks per sub-gather (keeps idx segments 64B-aligned)
            reg_vals = set()
            for t in range(NT):
                for q0 in range(0, NCHE[t], QG):
                    reg_vals.add(min(QG, NCHE[t] - q0) * P)
            ept_regs = {}
            for v in sorted(reg_vals):
                r = nc.alloc_registers(name=f"ept{v}")
                nc.regs_mov(r, v)
                ept_regs[v] = nc.snap(r, donate=False)



            # ============ node phase (own 2500 rows only) ============
            def node_phase(l, src_dram, gstat):
                cfg = LAYERS[l]
                Fin, C, R2 = cfg["Fin"], cfg["C"], cfg["R2"]
                HC = HEADS * C
                scale_off = None
                if cfg["bn"]:
                    sg = wpool.tile([128, 2], f32, tag="sg", name="sg")
                    nc.sync.dma_start(out=sg[:], in_=gstat[l - 1][:])
                    mu = wpool.tile([Fin, 1], f32, tag="mu", name="mu")
                    nc.vector.tensor_scalar(mu[:], sg[:, 0:1], 1.0 / N, None, OP.mult)
                    msq = wpool.tile([Fin, 1], f32, tag="msq", name="msq")
                    nc.vector.tensor_scalar(msq[:], sg[:, 1:2], 1.0 / N, None, OP.mult)
                    var = wpool.tile([Fin, 1], f32, tag="var", name="var")
                    nc.vector.tensor_tensor(var[:], mu[:], mu[:], OP.mult)
                    nc.vector.tensor_tensor(var[:], msq[:], var[:], OP.subtract)
                    nc.vector.tensor_scalar(var[:], var[:], BN_EPS, None, OP.add)
                    sdv = wpool.tile([Fin, 1], f32, tag="sdv", name="sdv")
                    nc.scalar.activation(sdv[:], var[:], AF.Sqrt)
                    rs = wpool.tile([Fin, 1], f32, tag="rs", name="rs")
                    nc.vector.reciprocal(rs[:], sdv[:])
                    bscale = wpool.tile([Fin, 1], f32, tag="bscale", name="bscale")
                    nc.vector.tensor_tensor(bscale[:], rs[:], bng_sb[l][:], OP.mult)
                    boff = wpool.tile([Fin, 1], f32, tag="boff", name="boff")
                    nc.vector.tensor_tensor(boff[:], mu[:], bscale[:], OP.mult)
                    nc.vector.tensor_tensor(boff[:], bnb_sb[l][:], boff[:], OP.subtract)
                    scale_off = (bscale, boff)

                for t in range(NT):
                    cnt = min(P, NPC - t * P)
                    xt = wpool.tile([P, Fin], f32, tag="xt", name="xt")
                    if cnt < P:
                        nc.vector.memset(xt[:], 0.0)
                    nc.sync.dma_start(out=xt[:cnt, :], in_=src_dram[t * P : t * P + cnt, :Fin])
                    xtp = ppool.tile([Fin, P], f32, tag="xtp", name="xtp", bufs=2)
                    nc.tensor.transpose(out=xtp[:], in_=xt[:], identity=ident_sb[:])
                    xts = wpool.tile([Fin, P], f32, tag="xts", name="xts")
                    if scale_off is not None:
                        # fused BN affine + relu on the Act engine
                        nc.scalar.activation(
                            xts[:], xtp[:], AF.Relu,
                            bias=scale_off[1][:], scale=scale_off[0][:],
                        )
                    else:
                        nc.scalar.activation(xts[:], xtp[:], AF.Identity)
                    tt = wpool.tile([P, HC + 8], bf16, tag="tt", name="tt")
                    sd = wpool.tile([P, 8], f32, tag="sd", name="sd")
                    if HC + 16 <= 512:
                        hp = ppool.tile([P, HC + 16], f32, tag="pmm", name="hp", bufs=2)
                        nc.tensor.matmul(out=hp[:], lhsT=xts[:], rhs=ww_sb[l][:], start=True, stop=True)
                        nc.vector.tensor_copy(tt[:], hp[:, : HC + 8])
                        nc.vector.tensor_copy(sd[:], hp[:, HC + 8 : HC + 16])
                    else:  # L4: 528 cols -> split 512 + 16
                        hp = ppool.tile([P, 512], f32, tag="pmm", name="hp", bufs=2)
                        hp2 = ppool.tile([P, 16], f32, tag="pmm2", name="hp2", bufs=1)
                        nc.tensor.matmul(out=hp[:], lhsT=xts[:], rhs=ww_sb[l][:, :512], start=True, stop=True)
                        nc.tensor.matmul(out=hp2[:], lhsT=xts[:], rhs=ww_sb[l][:, 512:], start=True, stop=True)
                        nc.vector.tensor_copy(tt[:, :512], hp[:])
                        nc.vector.tensor_copy(tt[:, 512:520], hp2[:, 0:8])
                        nc.vector.tensor_copy(sd[:], hp2[:, 8:16])
                    nc.sync.dma_start(
                        out=tabL[l][t * P : t * P + cnt, : HC + 8], in_=tt[:cnt, :]
                    )
                    nc.sync.dma_start(out=sdo[t * P : t * P + cnt, :], in_=sd[:cnt, :])

            # ============ edge phase (own dst tiles) ============
            def edge_phase(l, tabA, out_dram):
                cfg = LAYERS[l]
                C, R2 = cfg["C"], cfg["R2"]
                HC = HEADS * C
                do_stat = l in (0, 2)
                if do_stat:
                    accS = wpool.tile([HC, 2], f32, tag="accS", name="accS")
                    nc.vector.memset(accS[:], 0.0)
                for t in range(NT):
                    NCH = NCHE[t]  # effective chunks (real edges only)
                    cnt = min(P, NPC - t * P)
                    # one-hot O for all chunks of this tile (single DVE op);
                    # pad edges have dstloc=999 -> all-zero column
                    Oall = gpool.tile([P, NCH * P], bf16, tag="Oall", name="Oall")
                    nc.vector.tensor_tensor(
                        Oall[:].rearrange("p (j f) -> p j f", f=P),
                        iotab_sb[:].unsqueeze(1).to_broadcast((P, NCH, P)),
                        dstloc_sb[:, COFF[t] : COFF[t] + NCH].unsqueeze(2).to_broadcast((P, NCH, P)),
                        OP.is_equal,
                    )
                    # own s_dst rows for this tile (local, static offset)
                    sdtf = wpool.tile([P, 8], f32, tag="sdtf", name="sdtf")
                    if cnt < P:
                        nc.vector.memset(sdtf[:], 0.0)
                    nc.sync.dma_start(out=sdtf[:cnt, :], in_=sdo[t * P : t * P + cnt, :])
                    sdtb = wpool.tile([P, 8], bf16, tag="sdtb", name="sdtb")
                    nc.vector.tensor_copy(sdtb[:], sdtf[:])
                    # per-chunk SD[e,h] = s_dst[dst(e),h] via matmul with the
                    # host-precomputed transposed one-hot (no PE transposes)
                    psSD = ppool.tile([P, NCH * 8], f32, tag="psSD", name="psSD", bufs=1)
                    for j in range(NCH):
                        blk = (COFF[t] + j) * P
                        nc.tensor.matmul(
                            out=psSD[:, j * 8 : (j + 1) * 8],
                            lhsT=otall_sb[:, blk : blk + P], rhs=sdtb[:],
                            start=True, stop=True,
                        )
                    # gather source rows [h | s_src] (bf16) from the global
                    # table, split into sub-gathers round-robined over the 4
                    # SWDGE queues so their transfers can overlap
                    G = gpool.tile([P, NCH * R2], bf16, tag="G", name="G")
                    if not no_gather:
                        for q0 in range(0, NCH, QG):
                            qn = min(QG, NCH - q0)
                            nc.gpsimd.dma_gather(
                                out_ap=G[:, q0 * R2 : (q0 + qn) * R2].rearrange(
                                    "p (j r) -> p j r", r=R2
                                ),
                                in_ap=tabA[l][:],
                                idxs_ap=isrc_sb[
                                    :,
                                    EOFF[t] // 16 + q0 * 8 : EOFF[t] // 16
                                    + (q0 + qn) * 8,
                                ],
                                num_idxs=qn * P,
                                num_idxs_reg=ept_regs[qn * P],
                                elem_size=R2,
                                single_packet=False,
                                queue_num=(t + q0 // QG) % 4,
                            )
                    else:
                        nc.vector.memset(G[:, :8], 0.0)
                    G3 = G[:].rearrange("p (j r) -> p j r", r=R2)
                    # batched e = LeakyReLU(s_src + s_dst); EXS = exp(e) (bf16)
                    EB = wpool.tile([P, NCH * 8], f32, tag="EB", name="EB")
                    nc.vector.tensor_tensor(
                        EB[:].rearrange("p (j r) -> p j r", r=8),
                        G3[:, :, HC : HC + 8],
                        psSD[:].rearrange("p (j r) -> p j r", r=8),
                        OP.add,
                    )
                    EB2 = wpool.tile([P, NCH * 8], f32, tag="EB2", name="EB2")
                    nc.vector.tensor_scalar(EB2[:], EB[:], NEG, None, OP.mult)
                    nc.vector.tensor_tensor(EB[:], EB[:], EB2[:], OP.max)
                    EXS = wpool.tile([P, NCH * 8], bf16, tag="EXS", name="EXS")
                    nc.scalar.activation(EXS[:], EB[:], AF.Exp)
                    # h *= exp(e) in place on G (one 4D DVE op for all chunks)
                    nc.vector.tensor_tensor(
                        G3[:, :, :HC].rearrange("p j (h c) -> p j h c", c=C),
                        G3[:, :, :HC].rearrange("p j (h c) -> p j h c", c=C),
                        EXS[:].rearrange("p (j h) -> p j h", h=8)
                        .unsqueeze(3).to_broadcast((P, NCH, 8, C)),
                        OP.mult,
                    )
                    # overwrite the s_src columns with exp(e) (denominator)
                    nc.scalar.activation(
                        G3[:, :, HC : HC + 8],
                        EXS[:].rearrange("p (j r) -> p j r", r=8),
                        AF.Identity,
                    )
                    # per-chunk weighted scatter straight out of G
                    if HC + 8 <= 512:
                        psA = ppool.tile([P, HC + 8], f32, tag="pmm", name="psA", bufs=2)
                        psB = None
                    else:
                        psA = ppool.tile([P, 512], f32, tag="pmm", name="psA", bufs=2)
                        psB = ppool.tile([P, 8], f32, tag="pmm2", name="psB", bufs=1)
                    for j in range(NCH):
                        if psB is None:
                            nc.tensor.matmul(
                                out=psA[:], lhsT=Oall[:, j * P : (j + 1) * P],
                                rhs=G3[:, j, : HC + 8],
                                start=(j == 0), stop=(j == NCH - 1),
                            )
                        else:
                            nc.tensor.matmul(
                                out=psA[:], lhsT=Oall[:, j * P : (j + 1) * P],
                                rhs=G3[:, j, :512],
                                start=(j == 0), stop=(j == NCH - 1),
                            )
                            nc.tensor.matmul(
                                out=psB[:], lhsT=Oall[:, j * P : (j + 1) * P],
                                rhs=G3[:, j, 512 : HC + 8],
                                start=(j == 0), stop=(j == NCH - 1),
                            )
                    den = psA[:, HC : HC + 8] if psB is None else psB[:]
                    rec = wpool.tile([P, 8], f32, tag="rec", name="rec")
                    nc.vector.tensor_scalar(rec[:], den, 1e-16, None, OP.add)
                    nc.vector.reciprocal(rec[:], rec[:])
                    res = wpool.tile([P, HC], f32, tag="res", name="res")
                    nc.vector.tensor_tensor(
                        res[:].rearrange("p (h c) -> p h c", h=HEADS),
                        psA[:, :HC].rearrange("p (h c) -> p h c", h=HEADS),
                        rec[:].unsqueeze(2).to_broadcast((P, HEADS, C)),
                        OP.mult,
                    )
                    if cfg["concat"]:
                        nc.vector.tensor_tensor(res[:], res[:], bb_sb[l][:], OP.add)
                        nc.sync.dma_start(
                            out=out_dram[t * P : t * P + cnt, :], in_=res[:cnt, :]
                        )
                        if do_stat:
                            sq = wpool.tile([P, HC], f32, tag="sq", name="sq")
                            nc.scalar.square(sq[:], res[:])
                            psS1 = ppool.tile([HC, 1], f32, tag="psS", name="psS1", bufs=2)
                            nc.tensor.matmul(
                                out=psS1[:], lhsT=res[:cnt, :], rhs=ones_sb[:cnt, :],
                                start=True, stop=True,
                            )
                            nc.vector.tensor_tensor(accS[:, 0:1], accS[:, 0:1], psS1[:], OP.add)
                            psS2 = ppool.tile([HC, 1], f32, tag="psS", name="psS2", bufs=2)
                            nc.tensor.matmul(
                                out=psS2[:], lhsT=sq[:cnt, :], rhs=ones_sb[:cnt, :],
                                start=True, stop=True,
                            )
                            nc.vector.tensor_tensor(accS[:, 1:2], accS[:, 1:2], psS2[:], OP.add)
                    else:
                        red = wpool.tile([P, C], f32, tag="red", name="red")
                        nc.vector.tensor_reduce(
                            red[:],
                            res[:].rearrange("p (h c) -> p c h", h=HEADS),
                            AX,
                            OP.add,
                        )
                        nc.vector.tensor_scalar(red[:], red[:], 1.0 / HEADS, None, OP.mult)
                        nc.vector.tensor_tensor(red[:], red[:], bb_sb[l][:, :C], OP.add)
                        nc.sync.dma_start(
                            out=out_dram[t * P : t * P + cnt, :], in_=red[:cnt, :]
                        )
                if do_stat:
                    nc.sync.dma_start(out=ostat[l][:], in_=accS[:])

            # ================= full pipeline =================
            for _rep in range(repeat_k):
                tabA, gstat = fresh_shared(_rep)
                # chain reps through out_ext so repeat_k>1 timing can't be
                # dead-code-eliminated (used for timing only; output = model^k)
                srcs = [xin if _rep == 0 else out_ext, own[0], own[1], own[2]]
                outs = [own[0], own[1], own[2], out_ext]
                for l in range(4):
                    node_phase(l, srcs[l], gstat)
                    if local_coll:
                        nc.sync.dma_start(out=tabA[l][0:NPC, :], in_=tabL[l][:, :])
                    else:
                        nc.gpsimd.collective_compute(
                            "AllGather",
                            mybir.AluOpType.bypass,
                            replica_groups=[list(range(M))],
                            ins=[tabL[l].opt()],
                            outs=[tabA[l].opt()],
                        )
                    edge_phase(l, tabA, outs[l])
                    if l in (0, 2):
                        if local_coll:
                            nc.sync.dma_start(out=gstat[l][:, :], in_=ostat[l][:, :])
                        else:
                            nc.gpsimd.collective_compute(
                                "AllReduce",
                                mybir.AluOpType.add,
                                replica_groups=[list(range(M))],
                                ins=[ostat[l].opt()],
                                outs=[gstat[l].opt()],
                            )
    if not nc.is_finalized():
        nc.finalize()
    return nc


def _pjrt_exec(nc, in_maps, time_reps=0):
    """Mirror of bass2jax.run_bass_via_pjrt multi-core path, holding the jitted
    executable so repeated executions can be wall-timed."""
    import time as _t
    import jax
    from jax.experimental.shard_map import shard_map
    from jax.sharding import Mesh, PartitionSpec
    from concourse import bass2jax as B, mybir as mb

    B.install_neuronx_cc_hook()
    n_cores = len(in_maps)
    partition_name = nc.partition_id_tensor.name if nc.partition_id_tensor else None
    in_names, out_names, out_avals, zero_outs = [], [], [], []
    for alloc in nc.m.functions[0].allocations:
        if not isinstance(alloc, mb.MemoryLocationSet):
            continue
        name = alloc.memorylocations[0].name
        if alloc.kind == "ExternalInput":
            if name != partition_name:
                in_names.append(name)
        elif alloc.kind == "ExternalOutput":
            out_names.append(name)
            shape = tuple(alloc.tensor_shape)
            dtype = mb.dt.np(alloc.dtype)
            out_avals.append(jax.core.ShapedArray(shape, dtype))
            zero_outs.append(np.zeros(shape, dtype))
    n_params = len(in_names)
    n_outs = len(out_avals)
    in_names.extend(out_names)
    if partition_name is not None:
        in_names.append(partition_name)
    donate = tuple(range(n_params, n_params + n_outs))

    def _body(*args):
        operands = list(args)
        if partition_name is not None:
            operands.append(B.partition_id_tensor())
        outs = B._bass_exec_p.bind(
            *operands,
            out_avals=tuple(out_avals),
            in_names=tuple(in_names),
            out_names=tuple(out_names),
            lowering_input_output_aliases=(),
            sim_require_finite=True,
            sim_require_nnan=True,
            nc=nc,
        )
        return tuple(outs)

    devices = jax.devices()[:n_cores]
    mesh = Mesh(np.asarray(devices), ("core",))
    in_specs = (PartitionSpec("core"),) * (n_params + n_outs)
    out_specs = (PartitionSpec("core"),) * len(out_names)
    sharded = jax.jit(
        shard_map(_body, mesh=mesh, in_specs=in_specs, out_specs=out_specs,
                  check_rep=False),
        donate_argnums=donate, keep_unused=True,
    )
    per_core = [[np.asarray(m_[nm]) for nm in in_names[:n_params]] for m_ in in_maps]
    concat_in = [
        np.concatenate([per_core[c][i] for c in range(n_cores)], axis=0)
        for i in range(n_params)
    ]
    from jax.sharding import NamedSharding
    shard = NamedSharding(mesh, PartitionSpec("core"))
    concat_in = [jax.device_put(a, shard) for a in concat_in]
    jax.block_until_ready(concat_in)

    def once():
        cz = [jax.device_put(np.zeros((n_cores * z.shape[0], *z.shape[1:]), z.dtype), shard)
              for z in zero_outs]
        jax.block_until_ready(cz)
        t0 = _t.perf_counter()
        out_arrs = sharded(*concat_in, *cz)
        jax.block_until_ready(out_arrs)
        return _t.perf_counter() - t0, out_arrs

    _, out_arrs = once()  # compile + first run
    times = []
    for _ in range(time_reps):
        dt, out_arrs = once()
        times.append(dt)
    res = [
        {nm: np.asarray(out_arrs[i]).reshape(n_cores, *out_avals[i].shape)[c]
         for i, nm in enumerate(out_names)}
        for c in range(n_cores)
    ]
    # free device buffers so back-to-back _run calls don't accumulate HBM use
    import gc

    for a in list(concat_in) + list(out_arrs):
        try:
            a.delete()
        except Exception:
            pass
    del concat_in, out_arrs
    gc.collect()
    return res, (min(times) if times else None)


def _run(inputs, trace=False, time_reps=0, repeat_k=1, no_coll=False, no_gather=False):
    NCHT, VALT, NCHE, edata = _preprocess(np.asarray(inputs["edge_index"]))
    consts = _host_consts(inputs)
    nc = _build(NCHT, VALT, NCHE, repeat_k=repeat_k, no_coll=no_coll,
                no_gather=no_gather)

    x = np.asarray(inputs["x"], dtype=np.float32)
    in_maps = []
    for m in range(M):
        d = dict(consts)
        d.update(edata[m])
        d["xin"] = np.ascontiguousarray(x[m * NPC : (m + 1) * NPC])
        in_maps.append(d)

    if time_reps > 0:
        results, best_s = _pjrt_exec(nc, in_maps, time_reps=time_reps)
    else:
        from concourse.bass_utils import run_bass_kernel_spmd

        res = run_bass_kernel_spmd(nc, in_maps, core_ids=list(range(M)))
        results, best_s = res.results, None
    outs = [np.asarray(results[m]["out"]) for m in range(M)]
    full = np.concatenate(outs, axis=0).astype(np.float32)
    return full, (None if best_s is None else int(best_s * 1e9))


def kernel(**inputs):
    out, _ = _run(inputs, trace=False)
    return out

